# revision 1
# baseline (speedup 1.0000x reference)
"""HGT (heterogeneous graph transformer) Bass kernel for Trainium2, 8 NeuronCores.

Strategy (dst-sharded edges):
  - Destination nodes (per type) are sharded over the 8 cores; softmax +
    segment-sum are core-local (no all-reduce).
  - Node projections are node-sharded; (k_r|v_r) tables are AllGathered (bf16).
  - Per-edge k_r/v_r rows come from one 128-row indirect DMA per 128-edge tile
    (the SWDGE descriptor-emission rate is the primary bottleneck lane).
  - q[dst] is expanded on the TensorEngine with a one-hot matmul (edges sorted
    by dst => q rows of a window are SBUF-local); segment softmax/sum are
    one-hot matmuls accumulating in PSUM per 128-dst window; max-subtraction
    is skipped (logits are O(0.1)).
  - a_rel/m_rel/p_rel/scale folded into effective weights on the host.
  - Layers alternate relation order so the next layer's first AllGather (and
    the dense out/node passes feeding it) overlap the second relation's edge
    phase; gelu runs bulk in-place on g_fm so the ACT table isn't thrashed
    between Exp and Gelu.
"""
import os
import sys

import numpy as np

try:
    import concourse  # noqa: F401
except ImportError:  # pragma: no cover
    sys.path.insert(0, "/opt/trn_rl_repo")

import ml_dtypes

import concourse.bacc as bacc
import concourse.bass as bass
import concourse.tile as tile
from concourse import mybir
from concourse.bass_utils import run_bass_kernel_spmd

f32 = mybir.dt.float32
bf16 = mybir.dt.bfloat16
i32 = mybir.dt.int32
AF = mybir.ActivationFunctionType
ALU = mybir.AluOpType
BF = ml_dtypes.bfloat16

FULL_CFG = dict(N=100000, E=500000, HID=128, H=4, D=32, L=2, NC=8)
USE_TTR = os.environ.get('TTR', '0') == '1'  # hangs trn2 HW in this env  # fuse qk-mul+per-head-reduce into tensor_tensor_reduce


def _blockdiag(a):  # a: [H, D, D] -> [H*D, H*D]
    H, D, _ = a.shape
    out = np.zeros((H * D, H * D), np.float32)
    for h in range(H):
        out[h * D:(h + 1) * D, h * D:(h + 1) * D] = a[h]
    return out


def host_prep(inputs, cfg):
    N, E, HID, H, D, L, NC = (cfg[k] for k in ("N", "E", "HID", "H", "D", "L", "NC"))
    NSH = N // NC
    W = (NSH + 127) // 128
    NP = W * 128

    ip = {k: np.asarray(v) for k, v in inputs.items()}
    rel_st = [0, 1]
    rel_dt = [1, 0]
    edges = [ip["edge_ui"], ip["edge_iu"]]

    # ---- effective weights ----
    scale = 1.0 / np.sqrt(D)
    W3 = np.zeros((L, 2, HID, 3 * HID), np.float32)  # (l, r): [Wk_eff|Wv_eff|Wq_eff(t=r)]
    b3 = np.zeros((L, 2, 3 * HID), np.float32)
    for l in range(L):
        for r in range(2):
            st, dt = rel_st[r], rel_dt[r]
            BDa = _blockdiag(ip["a_rel"][l, r])
            BDm = _blockdiag(ip["m_rel"][l, r])
            W3[l, r, :, 0:HID] = ip["Wk"][l, st] @ BDa
            b3[l, r, 0:HID] = ip["bk"][l, st] @ BDa
            W3[l, r, :, HID:2 * HID] = ip["Wv"][l, st] @ BDm
            b3[l, r, HID:2 * HID] = ip["bv"][l, st] @ BDm
        for t in range(2):
            r_of = 1 - t  # relation whose dst type is t
            pscale = np.repeat(ip["p_rel"][l, r_of] * scale, D)
            W3[l, t, :, 2 * HID:3 * HID] = ip["Wq"][l, t] * pscale[None, :]
            b3[l, t, 2 * HID:3 * HID] = ip["bq"][l, t] * pscale
    beta = 1.0 / (1.0 + np.exp(-ip["skip"]))  # [L, T]

    # ---- edge schedules (identical across cores) ----
    def prep_rel(e):
        src, dst = e[0].astype(np.int64), e[1].astype(np.int64)
        gsrc = (src // NSH) * NP + (src % NSH)
        per_core = []
        counts = np.zeros((NC, W), np.int64)
        for c in range(NC):
            sel = (dst // NSH) == c
            s_c = gsrc[sel]
            dl_c = dst[sel] - c * NSH
            order = np.argsort(dl_c, kind="stable")
            s_c, dl_c = s_c[order], dl_c[order]
            counts[c] = np.bincount(dl_c // 128, minlength=W)
            per_core.append((s_c, dl_c))
        tiles_w = np.maximum(1, (counts.max(axis=0) + 127) // 128)
        NT = int(tiles_w.sum())
        idx_src = np.zeros((NC, NT * 128), np.int32)
        dloc = np.full((NC, NT * 128), 128.0, np.float32)  # 128 => pad slot
        for c in range(NC):
            s_c, dl_c = per_core[c]
            starts = np.concatenate([[0], np.cumsum(counts[c])])
            slot = 0
            for w in range(W):
                n = int(counts[c][w])
                a, b = int(starts[w]), int(starts[w]) + n
                idx_src[c, slot:slot + n] = s_c[a:b]
                dloc[c, slot:slot + n] = (dl_c[a:b] % 128).astype(np.float32)
                slot += int(tiles_w[w]) * 128
        return tiles_w, NT, idx_src, dloc

    schedules = []
    meta = []
    for r in range(2):
        tiles_w, NT, idx_src, dloc = prep_rel(edges[r])
        schedules.append((tiles_w, NT))
        meta.append((idx_src, dloc))

    # ---- per-core input arrays ----
    NTtot = schedules[0][1] + schedules[1][1]
    xs = [ip["x_user"].astype(np.float32), ip["x_item"].astype(np.float32)]
    in_maps = []
    for c in range(NC):
        x_fm = np.zeros((2, HID, NP), np.float32)
        for t in range(2):
            x_fm[t, :, :NSH] = xs[t][c * NSH:(c + 1) * NSH].T
        idx_cat = np.concatenate(
            [meta[0][0][c], meta[1][0][c]]).reshape(NTtot, 128).T
        dl = np.concatenate([meta[0][1][c], meta[1][1][c]])
        dloc_col = dl.reshape(NTtot, 128).T.astype(np.float32).copy()
        in_maps.append({
            "x_fm": x_fm,
            "idx_src": np.ascontiguousarray(idx_cat.astype(np.int32)),
            "dloc_col": np.ascontiguousarray(dloc_col),
        })

    # bias cols [128, NB] f32: 0,1 = b_in; 2.. = beta*bo (l,t)
    b_list = [ip["b_in"][0], ip["b_in"][1]]
    for l in range(L):
        for t in range(2):
            b_list.append(beta[l, t] * ip["bo"][l, t])
    Bcols = np.stack(b_list).astype(np.float32)

    bias_nz = [[bool(np.any(b3[l, r] != 0)) for r in range(2)] for l in range(L)]
    consts = {
        "bias_nz": bias_nz,
        "iota_row": np.tile(np.arange(128, dtype=np.float32), (128, 1)).astype(BF),
        "ident": np.eye(128, dtype=np.float32).astype(BF),
        "W3": W3.reshape(L * 2, HID, 3 * HID),
        "Win": ip["W_in"].astype(np.float32),
        "Wo_bf": ip["Wo"].astype(np.float32).reshape(L * 2, HID, HID).astype(BF),
        "b3": b3.reshape(1, L * 2 * 3 * HID).astype(np.float32),
        "Bcols": Bcols,
        "ones1f": np.ones((1, 128), np.float32),
    }
    dims = dict(NSH=NSH, W=W, NP=NP, NTtot=NTtot)
    return in_maps, consts, bases_dummy(), schedules, dims, beta


def bases_dummy():
    return {}


def build_program(cfg, consts, bases, schedules, dims, beta, sim_gelu=False):
    N, E, HID, H, D, L, NC = (cfg[k] for k in ("N", "E", "HID", "H", "D", "L", "NC"))
    NSH, W, NP, NTtot = dims["NSH"], dims["W"], dims["NP"], dims["NTtot"]
    NPALL = NP * NC
    rel_dt = [1, 0]
    NB = consts["Bcols"].shape[0]
    CHD = 448 if NP % 448 == 0 else 128      # dense (psum) chunk width
    CHN = 896 if NP % 896 == 0 else (512 if NP % 512 == 0 else NP)  # node h chunk
    assert NP % CHD == 0 and NP % CHN == 0 and CHN % 128 == 0

    nc = bacc.Bacc("TRN2", target_bir_lowering=False, debug=False, num_devices=NC)

    x_fm = nc.dram_tensor("x_fm", [2, HID, NP], f32, kind="ExternalInput")
    idx_src = nc.dram_tensor("idx_src", [128, NTtot], i32, kind="ExternalInput")
    dloc_col_d = nc.dram_tensor("dloc_col", [128, NTtot], f32, kind="ExternalInput")
    it_row_d = nc.dram_tensor("iota_row", [128, 128], bf16, kind="ExternalInput")
    ident_d = nc.dram_tensor("ident", [128, 128], bf16, kind="ExternalInput")
    W3_d = nc.dram_tensor("W3", [L * 2, HID, 3 * HID], f32, kind="ExternalInput")
    Win_d = nc.dram_tensor("Win", [2, HID, HID], f32, kind="ExternalInput")
    Wo_d = nc.dram_tensor("Wo_bf", [L * 2, HID, HID], bf16, kind="ExternalInput")
    b3_d = nc.dram_tensor("b3", [1, L * 2 * 3 * HID], f32, kind="ExternalInput")
    Bcols_d = nc.dram_tensor("Bcols", [NB, HID], f32, kind="ExternalInput")
    ones1f_d = nc.dram_tensor("ones1f", [1, 128], f32, kind="ExternalInput")
    out_d = nc.dram_tensor("out", [2, HID, NP], f32, kind="ExternalOutput")

    with tile.TileContext(nc) as tc:
        with tc.tile_pool(name="persist", bufs=1) as pp, \
             tc.tile_pool(name="dram", bufs=1, space="DRAM") as dp, \
             tc.tile_pool(name="wk_sb", bufs=3) as sb3, \
             tc.tile_pool(name="wk_sb2", bufs=2) as sb2, \
             tc.tile_pool(name="gath", bufs=16) as gpool, \
             tc.tile_pool(name="edge8", bufs=8) as sb8, \
             tc.tile_pool(name="ps_edge", bufs=2, space="PSUM") as ps_e, \
             tc.tile_pool(name="ps_dense", bufs=2, space="PSUM") as ps_d:

            # --- persistent SBUF ---
            it_row = pp.tile([128, 128], bf16)
            nc.sync.dma_start(it_row[:], it_row_d[:])
            ident = pp.tile([128, 128], bf16)
            nc.sync.dma_start(ident[:], ident_d[:])
            onesf = pp.tile([1, 128], f32)
            nc.sync.dma_start(onesf[:], ones1f_d[:])
            idxs = pp.tile([128, NTtot], i32)
            nc.sync.dma_start(idxs[:], idx_src[:])
            dloc_col = pp.tile([128, NTtot], f32)
            nc.sync.dma_start(dloc_col[:], dloc_col_d[:])
            w3sb = pp.tile([128, L * 2, 3 * HID], f32)
            nc.sync.dma_start(w3sb[:], W3_d[:].rearrange("k p d -> p k d"))
            winsb = pp.tile([128, 2, HID], f32)
            nc.sync.dma_start(winsb[:], Win_d[:].rearrange("k p d -> p k d"))
            wosb = pp.tile([128, L * 2, HID], bf16)
            nc.sync.dma_start(wosb[:], Wo_d[:].rearrange("k p d -> p k d"))
            b3sb = pp.tile([1, L * 2 * 3 * HID], f32)
            nc.sync.dma_start(b3sb[:], b3_d[:])
            bcols = pp.tile([128, NB], f32)
            nc.sync.dma_start(bcols[:], Bcols_d[:].rearrange("k d -> d k"))

            q_sb = [pp.tile([128, W, 128], bf16, name=f"q_sb{t}") for t in range(2)]
            g_fm = [pp.tile([128, NP], bf16, name=f"g_fm{t}") for t in range(2)]

            hA = [dp.tile([128, NP], f32, name=f"hA{t}") for t in range(2)]
            hB = [dp.tile([128, NP], f32, name=f"hB{t}") for t in range(2)]
            kvloc = [dp.tile([NP, 256], bf16, name=f"kvloc{r}") for r in range(2)]
            kvfull = [[dp.tile([NPALL, 256], bf16, name=f"kvfull{l}{r}")
                       for r in range(2)] for l in range(L)]
            rg = [list(range(NC))]

            def b3row(l, r, lo, hi):  # bias row slice [1, hi-lo]
                base = (l * 2 + r) * 3 * HID
                return b3sb[:, base + lo:base + hi]

            # dense projection pass over the node shard, writing kv and/or q
            def node_pass(l, r, h_src, do_kv, do_q):
                lo = 0 if do_kv else 2 * HID
                hi = 3 * HID if do_q else 2 * HID
                ncols = hi - lo
                for jc in range(NP // CHN):
                    hch = sb3.tile([128, CHN], f32, tag="hch")
                    nc.sync.dma_start(hch[:], h_src[:, jc * CHN:(jc + 1) * CHN])
                    for k in range(CHN // 128):
                        w = jc * (CHN // 128) + k
                        ps = ps_d.tile([128, 3 * HID], f32, tag="dense")
                        bias_nz = consts["bias_nz"][l][r]
                        nc.tensor.matmul(
                            out=ps[:, :ncols], lhsT=hch[:, k * 128:(k + 1) * 128],
                            rhs=w3sb[:, l * 2 + r, lo:hi], start=True,
                            stop=not bias_nz)
                        if bias_nz:
                            nc.tensor.matmul(
                                out=ps[:, :ncols], lhsT=onesf[:],
                                rhs=b3row(l, r, lo, hi), start=False, stop=True)
                        if do_kv:
                            kvt = sb3.tile([128, 256], bf16, tag="kvt")
                            nc.scalar.activation(kvt[:], ps[:, 0:256], AF.Copy)
                            nc.sync.dma_start(
                                kvloc[r][w * 128:(w + 1) * 128, :], kvt[:])
                        if do_q:
                            nc.vector.tensor_copy(
                                q_sb[r][:, w, :], ps[:, ncols - HID:ncols])

            def edge_phase(l, r, tbase):
                tiles_w, NT = schedules[r]
                dt = rel_dt[r]
                t_idx = tbase
                for w in range(W):
                    nt = int(tiles_w[w])
                    pswin = ps_e.tile([128, 132], f32, tag="win")
                    groups = []
                    k = 0
                    while k < nt:
                        g = min(2, nt - k)
                        groups.append((k, g))
                        k += g
                    for (k0, G) in groups:
                        tts = [t_idx + k0 + i for i in range(G)]
                        kvg = gpool.tile([128, 2, 256], bf16, tag="g")
                        for i, tt in enumerate(tts):
                            nc.gpsimd.indirect_dma_start(
                                out=kvg[:, i, :], out_offset=None,
                                in_=kvfull[l][r][:],
                                in_offset=bass.IndirectOffsetOnAxis(
                                    ap=idxs[:, tt:tt + 1], axis=0))
                        S2 = sb8.tile([128, 2, 128], bf16, tag="S")
                        for i, tt in enumerate(tts):
                            nc.vector.tensor_scalar(
                                out=S2[:, i, :], in0=it_row[:],
                                scalar1=dloc_col[:, tt:tt + 1], scalar2=None,
                                op0=ALU.is_equal)
                        psst = ps_e.tile([128, 2, 128], bf16, tag="st")
                        for i in range(G):
                            nc.tensor.transpose(out=psst[:, i, :], in_=S2[:, i, :],
                                                identity=ident[:])
                        St2 = sb8.tile([128, 2, 128], bf16, tag="St")
                        nc.vector.tensor_copy(St2[:, :G, :], psst[:, :G, :])
                        psqe = ps_e.tile([128, 2, 128], f32, tag="qe")
                        for i in range(G):
                            nc.tensor.matmul(out=psqe[:, i, :], lhsT=St2[:, i, :],
                                             rhs=q_sb[dt][:, w, :],
                                             start=True, stop=True)
                        qk = sb8.tile([128, 2, 128], f32, tag="qk")
                        nc.vector.tensor_tensor(
                            out=qk[:, :G, :].rearrange("p g (h d) -> p g h d", h=H),
                            in0=psqe[:, :G, :].rearrange("p g (h d) -> p g h d", h=H),
                            in1=kvg[:, :G, 0:128].rearrange("p g (h d) -> p g h d", h=H),
                            op=ALU.mult)
                        lg = sb8.tile([128, 2, 4], f32, tag="lg")
                        nc.vector.tensor_reduce(
                            out=lg[:, :G, :],
                            in_=qk[:, :G, :].rearrange("p g (h d) -> p (g h) d", h=H),
                            axis=mybir.AxisListType.X, op=ALU.add)
                        pay = sb8.tile([128, 2, 132], bf16, tag="pay")
                        nc.scalar.activation(pay[:, :G, 128:132], lg[:, :G, :], AF.Exp)
                        nc.vector.tensor_tensor(
                            out=pay[:, :G, 0:128].rearrange("p g (h d) -> p g h d", h=H),
                            in0=kvg[:, :G, 128:256].rearrange("p g (h d) -> p g h d", h=H),
                            in1=pay[:, :G, 128:132].to_broadcast([128, G, H, D]),
                            op=ALU.mult)
                        for i in range(G):
                            ki = k0 + i
                            nc.tensor.matmul(out=pswin[:], lhsT=S2[:, i, :],
                                             rhs=pay[:, i, :],
                                             start=(ki == 0), stop=(ki == nt - 1))
                    t_idx += nt
                    # window flush: normalize agg, transpose into g_fm
                    zrw = sb8.tile([128, 4], f32, tag="zrw")
                    nc.vector.tensor_scalar(out=zrw[:], in0=pswin[:, 128:132],
                                            scalar1=1e-16, scalar2=None, op0=ALU.add)
                    nc.vector.reciprocal(zrw[:], zrw[:])
                    gt = sb8.tile([128, 128], bf16, tag="gt")
                    nc.vector.tensor_tensor(
                        out=gt[:].rearrange("p (h d) -> p h d", h=H),
                        in0=pswin[:, 0:128].rearrange("p (h d) -> p h d", h=H),
                        in1=zrw[:].to_broadcast([128, H, D]),
                        op=ALU.mult)
                    psgt = ps_e.tile([128, 128], bf16, tag="st")
                    nc.tensor.transpose(out=psgt[:], in_=gt[:], identity=ident[:])
                    nc.vector.tensor_copy(g_fm[dt][:, w * 128:(w + 1) * 128], psgt[:])

            def bulk_gelu(t, lo, hi):
                if not sim_gelu:
                    nc.scalar.activation(g_fm[t][:, lo:hi], g_fm[t][:, lo:hi],
                                         AF.Gelu)
                else:
                    tmp = sb2.tile([128, NP], f32, tag="sgl")
                    g = g_fm[t][:, lo:hi]
                    tm = tmp[:, lo:hi]
                    nc.vector.tensor_tensor(out=tm, in0=g, in1=g, op=ALU.mult)
                    nc.vector.tensor_scalar(out=tm, in0=tm, scalar1=0.044715,
                                            scalar2=1.0, op0=ALU.mult, op1=ALU.add)
                    nc.vector.tensor_tensor(out=tm, in0=tm, in1=g, op=ALU.mult)
                    nc.scalar.activation(tm, tm, AF.Tanh, scale=0.7978845608028654)
                    nc.vector.tensor_scalar(out=tm, in0=tm, scalar1=1.0, scalar2=0.5,
                                            op0=ALU.add, op1=ALU.mult)
                    nc.vector.tensor_tensor(out=g, in0=tm, in1=g, op=ALU.mult)

            def out_phase(l, t, h_src, dst):
                bb = 2 + l * 2 + t
                coef = float((1.0 - beta[l, t]) + (1.0 if l > 0 else 0.0))
                bulk_gelu(t, 0, NP)
                for j in range(NP // CHD):
                    sl = slice(j * CHD, (j + 1) * CHD)
                    ps = ps_d.tile([128, CHD], f32, tag="dense")
                    nc.tensor.matmul(out=ps[:], lhsT=wosb[:, l * 2 + t, :],
                                     rhs=g_fm[t][:, sl], start=True, stop=True)
                    a1 = sb2.tile([128, CHD], f32, tag="a1")
                    nc.vector.tensor_scalar(
                        out=a1[:], in0=ps[:], scalar1=float(beta[l, t]),
                        scalar2=bcols[:, bb:bb + 1], op0=ALU.mult, op1=ALU.add)
                    hch = sb2.tile([128, CHD], f32, tag="hcho")
                    nc.sync.dma_start(hch[:], h_src[:, sl])
                    hn = sb2.tile([128, CHD], f32, tag="hn")
                    nc.vector.scalar_tensor_tensor(
                        out=hn[:], in0=hch[:], scalar=coef, in1=a1[:],
                        op0=ALU.mult, op1=ALU.add)
                    nc.sync.dma_start(dst[:, sl], hn[:])

            def input_proj(t, dst):
                for j in range(NP // CHD):
                    sl = slice(j * CHD, (j + 1) * CHD)
                    xt = sb2.tile([128, CHD], f32, tag="xt")
                    nc.sync.dma_start(xt[:], x_fm[t, :, sl])
                    ps = ps_d.tile([128, CHD], f32, tag="dense")
                    nc.tensor.matmul(out=ps[:], lhsT=winsb[:, t, :], rhs=xt[:],
                                     start=True, stop=True)
                    ht = sb2.tile([128, CHD], f32, tag="ht")
                    nc.scalar.activation(ht[:], ps[:], AF.Relu,
                                         bias=bcols[:, t:t + 1], scale=1.0)
                    nc.sync.dma_start(dst[:, sl], ht[:])

            # ---------------- schedule ----------------
            # layer l relation order alternates so the dense chain for the next
            # layer's first AG overlaps the current second edge phase.
            rorder = [[0, 1], [1, 0]][: L] if L <= 2 else None
            if L > 2:
                rorder = [[0, 1] if l % 2 == 0 else [1, 0] for l in range(L)]
            tb = [0, schedules[0][1]]  # tile base per relation

            h_cur = hA
            # layer 0 dense, interleaved with input projections
            rF, rS = rorder[0]
            input_proj(rF, hA[rF])
            node_pass(0, rF, hA[rF], do_kv=True, do_q=False)
            nc.gpsimd.collective_compute("AllGather", ALU.bypass, replica_groups=rg,
                                         ins=[kvloc[rF][:]], outs=[kvfull[0][rF][:]])
            input_proj(rS, hA[rS])
            node_pass(0, rS, hA[rS], do_kv=True, do_q=True)
            nc.gpsimd.collective_compute("AllGather", ALU.bypass, replica_groups=rg,
                                         ins=[kvloc[rS][:]], outs=[kvfull[0][rS][:]])
            node_pass(0, rF, hA[rF], do_kv=False, do_q=True)

            for l in range(L):
                rF, rS = rorder[l]
                last = l == L - 1
                h_nxt = hB if l == 0 else None
                # edge rF -> g_fm[dt(rF)]
                edge_phase(l, rF, tb[rF])
                # dense chain that only depends on edge rF:
                tF_out = rel_dt[rF]
                dstF = (h_nxt[tF_out] if not last else out_d[tF_out])
                out_phase(l, tF_out, h_cur[tF_out], dstF)
                if not last:
                    l2 = l + 1
                    rF2, rS2 = rorder[l2]
                    # node passes for next layer that depend only on h_nxt[tF_out]
                    # rF2 == tF_out by construction of alternating order
                    node_pass(l2, rF2, h_nxt[rF2], do_kv=True, do_q=False)
                    nc.gpsimd.collective_compute(
                        "AllGather", ALU.bypass, replica_groups=rg,
                        ins=[kvloc[rF2][:]], outs=[kvfull[l2][rF2][:]])
                # edge rS
                edge_phase(l, rS, tb[rS])
                tS_out = rel_dt[rS]
                dstS = (h_nxt[tS_out] if not last else out_d[tS_out])
                out_phase(l, tS_out, h_cur[tS_out], dstS)
                if not last:
                    node_pass(l2, rS2, h_nxt[rS2], do_kv=True, do_q=True)
                    nc.gpsimd.collective_compute(
                        "AllGather", ALU.bypass, replica_groups=rg,
                        ins=[kvloc[rS2][:]], outs=[kvfull[l2][rS2][:]])
                    node_pass(l2, rF2, h_nxt[rF2], do_kv=False, do_q=True)
                    h_cur = hB

    nc.finalize()
    return nc


def run(inputs, cfg=None, trace=False, trace_cores=None, sim=False):
    cfg = cfg or FULL_CFG
    NC = cfg["NC"]
    core_maps, consts, bases, schedules, dims, beta = host_prep(inputs, cfg)
    nc = build_program(cfg, consts, bases, schedules, dims, beta, sim_gelu=sim)
    in_maps = []
    for c in range(NC):
        m = dict(core_maps[c])
        for k in ("iota_row", "ident", "W3", "Win", "Wo_bf", "b3", "Bcols",
                  "ones1f"):
            m[k] = consts[k]
        in_maps.append(m)
    if sim:
        from concourse.bass_interp import MultiCoreSim

        msim = MultiCoreSim(nc, num_cores=NC, trace=False,
                            require_finite=False, require_nnan=False)
        cores = [msim.cores[c] for c in range(NC)]
        for c in range(NC):
            for name, arr in in_maps[c].items():
                cores[c].tensor(name)[:] = arr
        msim.simulate(check_with_hw=False)

        class R:
            exec_time_ns = None
            results = [{"out": np.asarray(cores[c].tensor("out"))}
                       for c in range(NC)]
        res = R()
    else:
        res = run_bass_kernel_spmd(nc, in_maps, core_ids=list(range(NC)),
                                   trace=trace, trace_cores=trace_cores)
    NSH, NP = dims["NSH"], dims["NP"]
    out = np.empty((2, cfg["N"], cfg["HID"]), np.float32)
    for c in range(NC):
        o = res.results[c]["out"]
        for t in range(2):
            out[t, c * NSH:(c + 1) * NSH] = o[t, :, :NSH].T
    return out, res


def kernel(**inputs):
    out, _ = run(inputs, FULL_CFG, trace=False)
    return out



# revision 6
# speedup vs baseline: 1.1570x; 1.1570x over previous
"""HGT Bass kernel for Trainium2, 8 NeuronCores — v3.

Dst-sharded edges with host-side balanced windows (~1% tile padding),
AllGathered bf16 kv tables, per-window segment softmax via one-hot matmuls.
Per-edge kv rows come from the proven per-tile [128,1] indirect DMA; the
per-edge q rows are expanded on the PE from SBUF-resident per-window q via
a DVE-built transposed one-hot (no PE transpose, no PSUM round-trip).
All dense projections run in bf16 (f32 matmuls are 4x slower on the PE).
"""
import sys

import numpy as np

try:
    import concourse  # noqa: F401
except ImportError:  # pragma: no cover
    sys.path.insert(0, "/opt/trn_rl_repo")

import ml_dtypes

import concourse.bacc as bacc
import concourse.bass as bass
import concourse.tile as tile
from concourse import mybir
from concourse.bass_utils import run_bass_kernel_spmd

f32 = mybir.dt.float32
bf16 = mybir.dt.bfloat16
i32 = mybir.dt.int32
AF = mybir.ActivationFunctionType
ALU = mybir.AluOpType
BF = ml_dtypes.bfloat16

FULL_CFG = dict(N=100000, E=500000, HID=128, H=4, D=32, L=2, NC=8)


def _blockdiag(a):
    H, D, _ = a.shape
    out = np.zeros((H * D, H * D), np.float32)
    for h in range(H):
        out[h * D:(h + 1) * D, h * D:(h + 1) * D] = a[h]
    return out


def _balance_windows(deg, W, cap=640):
    """LPT-balance nodes into W windows of <=128 slots, then push excess
    above `cap` edges into the single overflow window W-1 via degree swaps."""
    import heapq
    n = len(deg)
    order = np.argsort(-deg, kind="stable")
    heap = [(0, w) for w in range(W)]
    heapq.heapify(heap)
    slots_used = np.zeros(W, np.int64)
    edge_cnt = np.zeros(W, np.int64)
    assign = np.empty(n, np.int64)
    for nd in order:
        while True:
            cnt, w = heapq.heappop(heap)
            if cnt == edge_cnt[w] and slots_used[w] < 128:
                break
        assign[nd] = w
        slots_used[w] += 1
        edge_cnt[w] += deg[nd]
        heapq.heappush(heap, (edge_cnt[w], w))
    ov = W - 1
    if W >= 2 and edge_cnt.max() > cap:
        nodes_w = [list(np.where(assign == w)[0]) for w in range(W)]
        for w in range(W):
            if w == ov:
                continue
            while edge_cnt[w] > cap:
                part = min(nodes_w[ov], key=lambda nd: deg[nd])
                need = edge_cnt[w] - cap + deg[part]
                cands = [nd for nd in nodes_w[w] if deg[nd] >= need]
                cand = (min(cands, key=lambda nd: deg[nd]) if cands
                        else max(nodes_w[w], key=lambda nd: deg[nd]))
                if deg[cand] <= deg[part]:
                    break
                nodes_w[w].remove(cand)
                nodes_w[ov].remove(part)
                nodes_w[w].append(part)
                nodes_w[ov].append(cand)
                edge_cnt[w] += deg[part] - deg[cand]
                edge_cnt[ov] += deg[cand] - deg[part]
        for w in range(W):
            for nd in nodes_w[w]:
                assign[nd] = w
    slot_in_w = np.zeros(n, np.int64)
    fill = np.zeros(W, np.int64)
    for nd in range(n):
        w = assign[nd]
        slot_in_w[nd] = fill[w]
        fill[w] += 1
    return assign * 128 + slot_in_w


def host_prep(inputs, cfg):
    N, E, HID, H, D, L, NC = (cfg[k] for k in ("N", "E", "HID", "H", "D", "L", "NC"))
    NSH = N // NC
    W = (NSH + 127) // 128
    NP = W * 128

    ip = {k: np.asarray(v) for k, v in inputs.items()}
    rel_st = [0, 1]
    rel_dt = [1, 0]
    edges = [ip["edge_ui"], ip["edge_iu"]]

    scale = 1.0 / np.sqrt(D)
    W3 = np.zeros((L, 2, HID, 3 * HID), np.float32)
    b3 = np.zeros((L, 2, 3 * HID), np.float32)
    for l in range(L):
        for r in range(2):
            st = rel_st[r]
            BDa = _blockdiag(ip["a_rel"][l, r])
            BDm = _blockdiag(ip["m_rel"][l, r])
            W3[l, r, :, 0:HID] = ip["Wk"][l, st] @ BDa
            b3[l, r, 0:HID] = ip["bk"][l, st] @ BDa
            W3[l, r, :, HID:2 * HID] = ip["Wv"][l, st] @ BDm
            b3[l, r, HID:2 * HID] = ip["bv"][l, st] @ BDm
        for t in range(2):
            r_of = 1 - t
            pscale = np.repeat(ip["p_rel"][l, r_of] * scale, D)
            W3[l, t, :, 2 * HID:3 * HID] = ip["Wq"][l, t] * pscale[None, :]
            b3[l, t, 2 * HID:3 * HID] = ip["bq"][l, t] * pscale
    beta = 1.0 / (1.0 + np.exp(-ip["skip"]))

    r_of_dt = [1, 0]
    newslot = np.zeros((2, NC, NSH), np.int64)
    for t in range(2):
        r = r_of_dt[t]
        dst = edges[r][1].astype(np.int64)
        deg_all = np.bincount(dst, minlength=N)
        for c in range(NC):
            newslot[t, c] = _balance_windows(deg_all[c * NSH:(c + 1) * NSH], W)
    orderv = np.full((2, NC, NP), -1, np.int64)
    for t in range(2):
        for c in range(NC):
            orderv[t, c, newslot[t, c]] = np.arange(NSH)

    def prep_rel(r):
        e = edges[r]
        st, dt = rel_st[r], rel_dt[r]
        src, dst = e[0].astype(np.int64), e[1].astype(np.int64)
        csrc = src // NSH
        gsrc = csrc * NP + newslot[st][csrc, src % NSH]
        counts = np.zeros((NC, W), np.int64)
        per_core = []
        for c in range(NC):
            sel = (dst // NSH) == c
            s_c = gsrc[sel]
            dl_c = newslot[dt, c][dst[sel] - c * NSH]
            order = np.argsort(dl_c, kind="stable")
            s_c, dl_c = s_c[order], dl_c[order]
            counts[c] = np.bincount(dl_c // 128, minlength=W)
            per_core.append((s_c, dl_c))
        tiles_w = np.maximum(1, (counts.max(axis=0) + 127) // 128)
        NT = int(tiles_w.sum())
        idx_src = np.zeros((NC, NT * 128), np.int32)
        dloc = np.full((NC, NT * 128), 128.0, np.float32)
        for c in range(NC):
            s_c, dl_c = per_core[c]
            starts = np.concatenate([[0], np.cumsum(counts[c])])
            slot = 0
            for w in range(W):
                n = int(counts[c][w])
                a, b = int(starts[w]), int(starts[w]) + n
                idx_src[c, slot:slot + n] = s_c[a:b]
                dloc[c, slot:slot + n] = (dl_c[a:b] % 128).astype(np.float32)
                slot += int(tiles_w[w]) * 128
        return tiles_w, NT, idx_src, dloc

    schedules, meta = [], []
    for r in range(2):
        tiles_w, NT, idx_src, dloc = prep_rel(r)
        schedules.append((tiles_w, NT))
        meta.append((idx_src, dloc))

    NTtot = schedules[0][1] + schedules[1][1]
    W2 = 2 * W
    NTWMAX = max(int(t) for s in schedules for t in s[0])
    xs = [ip["x_user"].astype(np.float32), ip["x_item"].astype(np.float32)]
    in_maps = []
    for c in range(NC):
        x_fm = np.zeros((2, HID, NP), np.float32)
        for t in range(2):
            x_fm[t][:, newslot[t, c]] = xs[t][c * NSH:(c + 1) * NSH].T
        idx_cat = np.concatenate(
            [meta[0][0][c], meta[1][0][c]]).reshape(NTtot, 128).T
        dl = np.concatenate([meta[0][1][c], meta[1][1][c]])
        dloc_col = dl.reshape(NTtot, 128).T.astype(BF).copy()
        in_maps.append({
            "x_fm": x_fm.astype(BF),
            "idx_src": np.ascontiguousarray(idx_cat.astype(np.int32)),
            "dloc_col": np.ascontiguousarray(dloc_col),
        })

    b_list = [ip["b_in"][0], ip["b_in"][1]]
    for l in range(L):
        for t in range(2):
            b_list.append(beta[l, t] * ip["bo"][l, t])
    Bcols = np.stack(b_list).astype(np.float32)

    bias_nz = [[bool(np.any(b3[l, r] != 0)) for r in range(2)] for l in range(L)]
    consts = {
        "bias_nz": bias_nz,
        "iota_row": np.tile(np.arange(128, dtype=np.float32), (128, 1)).astype(BF),
        "ident": np.eye(128, dtype=np.float32).astype(BF),
        "W3": W3.reshape(L * 2, HID, 3 * HID).astype(BF),
        "Win": ip["W_in"].astype(np.float32).astype(BF),
        "Wo_bf": ip["Wo"].astype(np.float32).reshape(L * 2, HID, HID).astype(BF),
        "b3": b3.reshape(1, L * 2 * 3 * HID).astype(np.float32),
        "Bcols": Bcols,
        "ones1f": np.ones((1, 128), np.float32).astype(BF),
    }
    dims = dict(NSH=NSH, W=W, NP=NP, NTtot=NTtot, NTWMAX=NTWMAX, W2=W2)
    return in_maps, consts, orderv, schedules, dims, beta


def build_program(cfg, consts, schedules, dims, beta, sim_gelu=False):
    N, E, HID, H, D, L, NC = (cfg[k] for k in ("N", "E", "HID", "H", "D", "L", "NC"))
    NSH, W, NP, NTtot = dims["NSH"], dims["W"], dims["NP"], dims["NTtot"]
    NTWMAX, W2 = dims["NTWMAX"], dims["W2"]
    NPALL = NP * NC
    rel_dt = [1, 0]
    NB = consts["Bcols"].shape[0]
    CHD = 448 if NP % 448 == 0 else 128
    CHN = 896 if NP % 896 == 0 else (512 if NP % 512 == 0 else NP)
    assert NP % CHD == 0 and NP % CHN == 0 and CHN % 128 == 0

    nc = bacc.Bacc("TRN2", target_bir_lowering=False, debug=False,
                   num_devices=NC)

    x_fm = nc.dram_tensor("x_fm", [2, HID, NP], bf16, kind="ExternalInput")
    idx_src = nc.dram_tensor("idx_src", [128, NTtot], i32, kind="ExternalInput")
    dloc_col_d = nc.dram_tensor("dloc_col", [128, NTtot], bf16,
                                kind="ExternalInput")
    it_row_d = nc.dram_tensor("iota_row", [128, 128], bf16, kind="ExternalInput")
    ident_d = nc.dram_tensor("ident", [128, 128], bf16, kind="ExternalInput")
    W3_d = nc.dram_tensor("W3", [L * 2, HID, 3 * HID], bf16, kind="ExternalInput")
    Win_d = nc.dram_tensor("Win", [2, HID, HID], bf16, kind="ExternalInput")
    Wo_d = nc.dram_tensor("Wo_bf", [L * 2, HID, HID], bf16, kind="ExternalInput")
    b3_d = nc.dram_tensor("b3", [1, L * 2 * 3 * HID], f32, kind="ExternalInput")
    Bcols_d = nc.dram_tensor("Bcols", [NB, HID], f32, kind="ExternalInput")
    ones1f_d = nc.dram_tensor("ones1f", [1, 128], bf16, kind="ExternalInput")
    out_d = nc.dram_tensor("out", [2, HID, NP], f32, kind="ExternalOutput")

    with tile.TileContext(nc) as tc:
        with tc.tile_pool(name="persist", bufs=1) as pp, \
             tc.tile_pool(name="dram", bufs=1, space="DRAM") as dp, \
             tc.tile_pool(name="wk_sb", bufs=3) as sb3, \
             tc.tile_pool(name="wk_sb2", bufs=2) as sb2, \
             tc.tile_pool(name="gath", bufs=3) as gpool, \
             tc.tile_pool(name="edge8", bufs=3) as sb8, \
             tc.tile_pool(name="ps_win", bufs=2, space="PSUM") as ps_w, \
             tc.tile_pool(name="ps_qe", bufs=2, space="PSUM") as ps_q, \
             tc.tile_pool(name="ps_dense", bufs=2, space="PSUM") as ps_d:

            it_row = pp.tile([128, 128], bf16)
            nc.sync.dma_start(it_row[:], it_row_d[:])
            ident = pp.tile([128, 128], bf16)
            nc.sync.dma_start(ident[:], ident_d[:])
            onesf = pp.tile([1, 128], bf16)
            nc.sync.dma_start(onesf[:], ones1f_d[:])
            idxs = pp.tile([128, NTtot], i32)
            nc.sync.dma_start(idxs[:], idx_src[:])
            dloc_col = pp.tile([128, NTtot], bf16)
            nc.sync.dma_start(dloc_col[:], dloc_col_d[:])
            w3sb = pp.tile([128, L * 2, 3 * HID], bf16)
            nc.sync.dma_start(w3sb[:], W3_d[:].rearrange("k p d -> p k d"))
            winsb = pp.tile([128, 2, HID], bf16)
            nc.sync.dma_start(winsb[:], Win_d[:].rearrange("k p d -> p k d"))
            wosb = pp.tile([128, L * 2, HID], bf16)
            nc.sync.dma_start(wosb[:], Wo_d[:].rearrange("k p d -> p k d"))
            b3sb = pp.tile([1, L * 2 * 3 * HID], f32)
            nc.sync.dma_start(b3sb[:], b3_d[:])
            bcols = pp.tile([128, NB], f32)
            nc.sync.dma_start(bcols[:], Bcols_d[:].rearrange("k d -> d k"))

            g_fm = [pp.tile([128, NP], bf16, name=f"g_fm{t}") for t in range(2)]
            q_sb = [pp.tile([128, W, 128], bf16, name=f"q_sb{t}")
                    for t in range(2)]

            hA = [dp.tile([128, NP], bf16, name=f"hA{t}") for t in range(2)]
            hB = [dp.tile([128, NP], bf16, name=f"hB{t}") for t in range(2)]
            kvloc = [dp.tile([NP, 256], bf16, name=f"kvloc{r}") for r in range(2)]
            kvfull = [[dp.tile([NPALL, 256], bf16, name=f"kvfull{l}{r}")
                       for r in range(2)] for l in range(L)]
            rg = [list(range(NC))]

            def b3row(l, r, lo, hi):
                base = (l * 2 + r) * 3 * HID
                return b3sb[:, base + lo:base + hi]

            def node_pass(l, r, h_src):
                for jc in range(NP // CHN):
                    hch = sb3.tile([128, CHN], bf16, tag="hch")
                    nc.sync.dma_start(hch[:], h_src[:, jc * CHN:(jc + 1) * CHN])
                    for k in range(CHN // 128):
                        w = jc * (CHN // 128) + k
                        ps = ps_d.tile([128, 3 * HID], f32, tag="dense")
                        bias_nz = consts["bias_nz"][l][r]
                        nc.tensor.matmul(
                            out=ps[:], lhsT=hch[:, k * 128:(k + 1) * 128],
                            rhs=w3sb[:, l * 2 + r, :], start=True,
                            stop=not bias_nz)
                        if bias_nz:
                            nc.tensor.matmul(
                                out=ps[:], lhsT=onesf[:],
                                rhs=b3row(l, r, 0, 3 * HID), start=False,
                                stop=True)
                        kvt = sb3.tile([128, 3 * HID], bf16, tag="kvt")
                        nc.scalar.activation(kvt[:], ps[:], AF.Copy)
                        nc.sync.dma_start(
                            kvloc[r][w * 128:(w + 1) * 128, :], kvt[:, 0:256])
                        nc.vector.tensor_copy(q_sb[r][:, w, :],
                                              kvt[:, 256:384])

            def edge_phase(l, r, tbase, wbase):
                tiles_w, NT = schedules[r]
                dt = rel_dt[r]
                t0 = tbase
                for w in range(W):
                    nt = int(tiles_w[w])
                    kvg = gpool.tile([128, NTWMAX, 256], bf16, tag="kv")
                    for i in range(nt):
                        nc.gpsimd.indirect_dma_start(
                            out=kvg[:, i, :], out_offset=None,
                            in_=kvfull[l][r][:],
                            in_offset=bass.IndirectOffsetOnAxis(
                                ap=idxs[:, t0 + i:t0 + i + 1], axis=0))
                    S2 = sb8.tile([128, NTWMAX, 128], bf16, tag="S")
                    nc.vector.tensor_tensor(
                        out=S2[:, 0:nt, :],
                        in0=it_row[:].unsqueeze(1).to_broadcast([128, nt, 128]),
                        in1=dloc_col[:, t0:t0 + nt].unsqueeze(2)
                            .to_broadcast([128, nt, 128]),
                        op=ALU.is_equal)
                    pay = sb8.tile([128, NTWMAX, 132], bf16, tag="pay")
                    k2 = 0
                    while k2 < nt:
                        g = min(2, nt - k2)
                        psst = ps_q.tile([128, 2, 128], bf16, tag="stt")
                        for i in range(g):
                            nc.tensor.transpose(out=psst[:, i, :],
                                                in_=S2[:, k2 + i, :],
                                                identity=ident[:])
                        St2 = sb8.tile([128, 2, 128], bf16, tag="St")
                        nc.vector.tensor_copy(St2[:, 0:g, :], psst[:, 0:g, :])
                        psqe = ps_q.tile([128, 2, 128], f32, tag="qe")
                        for i in range(g):
                            nc.tensor.matmul(out=psqe[:, i, :],
                                             lhsT=St2[:, i, :],
                                             rhs=q_sb[dt][:, w, :],
                                             start=True, stop=True)
                        qk = sb8.tile([128, 2, 128], bf16, tag="qk")
                        nc.vector.tensor_tensor(
                            out=qk[:, 0:g, :], in0=psqe[:, 0:g, :],
                            in1=kvg[:, k2:k2 + g, 0:128], op=ALU.mult)
                        lg = sb8.tile([128, 2, H], f32, tag="lg")
                        nc.vector.tensor_reduce(
                            out=lg[:, 0:g, :],
                            in_=qk[:, 0:g, :].rearrange(
                                "p g (h d) -> p (g h) d", h=H),
                            axis=mybir.AxisListType.X, op=ALU.add)
                        nc.scalar.activation(pay[:, k2:k2 + g, 128:132],
                                             lg[:, 0:g, :], AF.Exp)
                        k2 += g
                    nc.vector.tensor_tensor(
                        out=pay[:, 0:nt, 0:128].rearrange(
                            "p g (h d) -> p g h d", h=H),
                        in0=kvg[:, 0:nt, 128:256].rearrange(
                            "p g (h d) -> p g h d", h=H),
                        in1=pay[:, 0:nt, 128:132].unsqueeze(3)
                            .to_broadcast([128, nt, H, D]),
                        op=ALU.mult)
                    pswin = ps_w.tile([128, 132], f32, tag="win")
                    for i in range(nt):
                        nc.tensor.matmul(out=pswin[:], lhsT=S2[:, i, :],
                                         rhs=pay[:, i, :],
                                         start=(i == 0), stop=(i == nt - 1))
                    zrw = sb8.tile([128, 4], f32, tag="zrw")
                    nc.vector.tensor_scalar(
                        out=zrw[:], in0=pswin[:, 128:132],
                        scalar1=1e-16, scalar2=None, op0=ALU.add)
                    nc.vector.reciprocal(zrw[:], zrw[:])
                    gt = sb8.tile([128, 128], bf16, tag="gt")
                    nc.vector.tensor_tensor(
                        out=gt[:].rearrange("p (h d) -> p h d", h=H),
                        in0=pswin[:, 0:128].rearrange("p (h d) -> p h d", h=H),
                        in1=zrw[:].unsqueeze(2).to_broadcast([128, H, D]),
                        op=ALU.mult)
                    psgt = ps_q.tile([128, 2, 128], bf16, tag="stt")
                    nc.tensor.transpose(out=psgt[:, 0, :], in_=gt[:],
                                        identity=ident[:])
                    nc.vector.tensor_copy(
                        g_fm[dt][:, w * 128:(w + 1) * 128], psgt[:, 0, :])
                    t0 += nt

            def bulk_gelu(t, lo, hi):
                if not sim_gelu:
                    nc.scalar.activation(g_fm[t][:, lo:hi], g_fm[t][:, lo:hi],
                                         AF.Gelu)
                else:
                    tmp = sb2.tile([128, NP], f32, tag="sgl")
                    g = g_fm[t][:, lo:hi]
                    tm = tmp[:, lo:hi]
                    nc.vector.tensor_tensor(out=tm, in0=g, in1=g, op=ALU.mult)
                    nc.vector.tensor_scalar(out=tm, in0=tm, scalar1=0.044715,
                                            scalar2=1.0, op0=ALU.mult,
                                            op1=ALU.add)
                    nc.vector.tensor_tensor(out=tm, in0=tm, in1=g, op=ALU.mult)
                    nc.scalar.activation(tm, tm, AF.Tanh,
                                         scale=0.7978845608028654)
                    nc.vector.tensor_scalar(out=tm, in0=tm, scalar1=1.0,
                                            scalar2=0.5, op0=ALU.add,
                                            op1=ALU.mult)
                    nc.vector.tensor_tensor(out=g, in0=tm, in1=g, op=ALU.mult)

            def out_phase(l, t, h_src, dst, last):
                bb = 2 + l * 2 + t
                coef = float((1.0 - beta[l, t]) + (1.0 if l > 0 else 0.0))
                bulk_gelu(t, 0, NP)
                for j in range(NP // CHD):
                    sl = slice(j * CHD, (j + 1) * CHD)
                    ps = ps_d.tile([128, CHD], f32, tag="dense")
                    nc.tensor.matmul(out=ps[:], lhsT=wosb[:, l * 2 + t, :],
                                     rhs=g_fm[t][:, sl], start=True, stop=True)
                    a1 = sb2.tile([128, CHD], f32, tag="a1")
                    nc.vector.tensor_scalar(
                        out=a1[:], in0=ps[:], scalar1=float(beta[l, t]),
                        scalar2=bcols[:, bb:bb + 1], op0=ALU.mult, op1=ALU.add)
                    hch = sb2.tile([128, CHD], bf16, tag="hcho")
                    nc.sync.dma_start(hch[:], h_src[:, sl])
                    hn = sb2.tile([128, CHD], f32 if last else bf16,
                                  tag="hnf" if last else "hn")
                    nc.vector.scalar_tensor_tensor(
                        out=hn[:], in0=hch[:], scalar=coef, in1=a1[:],
                        op0=ALU.mult, op1=ALU.add)
                    nc.sync.dma_start(dst[:, sl], hn[:])

            def input_proj(t, dst):
                for j in range(NP // CHD):
                    sl = slice(j * CHD, (j + 1) * CHD)
                    xt = sb2.tile([128, CHD], bf16, tag="xt")
                    nc.sync.dma_start(xt[:], x_fm[t, :, sl])
                    ps = ps_d.tile([128, CHD], f32, tag="dense")
                    nc.tensor.matmul(out=ps[:], lhsT=winsb[:, t, :], rhs=xt[:],
                                     start=True, stop=True)
                    ht = sb2.tile([128, CHD], bf16, tag="ht")
                    nc.scalar.activation(ht[:], ps[:], AF.Relu,
                                         bias=bcols[:, t:t + 1], scale=1.0)
                    nc.sync.dma_start(dst[:, sl], ht[:])

            # ---------------- schedule ----------------
            rorder = [[0, 1] if l % 2 == 0 else [1, 0] for l in range(L)]
            tb = [0, schedules[0][1]]
            wb = [0, W]

            h_cur = hA
            rF, rS = rorder[0]
            input_proj(rF, hA[rF])
            node_pass(0, rF, hA[rF])
            nc.gpsimd.collective_compute(
                "AllGather", ALU.bypass, replica_groups=rg,
                ins=[kvloc[rF][:]], outs=[kvfull[0][rF][:]])
            input_proj(rS, hA[rS])
            node_pass(0, rS, hA[rS])
            nc.gpsimd.collective_compute(
                "AllGather", ALU.bypass, replica_groups=rg,
                ins=[kvloc[rS][:]], outs=[kvfull[0][rS][:]])

            for l in range(L):
                rF, rS = rorder[l]
                last = l == L - 1
                h_nxt = hB if l == 0 else None
                edge_phase(l, rF, tb[rF], wb[rF])
                tF_out = rel_dt[rF]
                dstF = (h_nxt[tF_out] if not last else out_d[tF_out])
                out_phase(l, tF_out, h_cur[tF_out], dstF, last)
                if not last:
                    l2 = l + 1
                    rF2, rS2 = rorder[l2]
                    node_pass(l2, rF2, h_nxt[rF2])
                    nc.gpsimd.collective_compute(
                        "AllGather", ALU.bypass, replica_groups=rg,
                        ins=[kvloc[rF2][:]], outs=[kvfull[l2][rF2][:]])
                edge_phase(l, rS, tb[rS], wb[rS])
                tS_out = rel_dt[rS]
                dstS = (h_nxt[tS_out] if not last else out_d[tS_out])
                out_phase(l, tS_out, h_cur[tS_out], dstS, last)
                if not last:
                    node_pass(l2, rS2, h_nxt[rS2])
                    nc.gpsimd.collective_compute(
                        "AllGather", ALU.bypass, replica_groups=rg,
                        ins=[kvloc[rS2][:]], outs=[kvfull[l2][rS2][:]])
                    h_cur = hB

    nc.finalize()
    return nc


def run(inputs, cfg=None, trace=False, trace_cores=None, sim=False):
    cfg = cfg or FULL_CFG
    NC = cfg["NC"]
    core_maps, consts, orderv, schedules, dims, beta = host_prep(inputs, cfg)
    nc = build_program(cfg, consts, schedules, dims, beta, sim_gelu=sim)
    in_maps = []
    for c in range(NC):
        m = dict(core_maps[c])
        for k in ("iota_row", "ident", "W3", "Win", "Wo_bf", "b3",
                  "Bcols", "ones1f"):
            m[k] = consts[k]
        in_maps.append(m)
    if sim:
        from concourse.bass_interp import MultiCoreSim

        msim = MultiCoreSim(nc, num_cores=NC, trace=False,
                            require_finite=False, require_nnan=False)
        cores = [msim.cores[c] for c in range(NC)]
        for c in range(NC):
            for name, arr in in_maps[c].items():
                cores[c].tensor(name)[:] = arr
        msim.simulate(check_with_hw=False)

        class R:
            exec_time_ns = None
            results = [{"out": np.asarray(cores[c].tensor("out"))}
                       for c in range(NC)]
        res = R()
    else:
        res = run_bass_kernel_spmd(nc, in_maps, core_ids=list(range(NC)),
                                   trace=trace, trace_cores=trace_cores)
    NSH, NP = dims["NSH"], dims["NP"]
    out = np.empty((2, cfg["N"], cfg["HID"]), np.float32)
    for c in range(NC):
        o = res.results[c]["out"]
        for t in range(2):
            sel = orderv[t, c] >= 0
            slots = np.where(sel)[0]
            orig = orderv[t, c][slots]
            out[t, c * NSH + orig] = o[t][:, slots].T
    return out, res


def kernel(**inputs):
    out, _ = run(inputs, FULL_CFG, trace=False)
    return out


# revision 7
# speedup vs baseline: 1.2054x; 1.0419x over previous
"""HGT Bass kernel for Trainium2, 8 NeuronCores — v5 (fp8 kv, streamed one-hots).

Dst-sharded edges with host-side balanced windows (~1% tile padding),
AllGathered bf16 kv tables, per-window segment softmax via one-hot matmuls.
Per-edge kv rows come from the proven per-tile [128,1] indirect DMA; the
per-edge q rows are expanded on the PE from SBUF-resident per-window q via
a DVE-built transposed one-hot (no PE transpose, no PSUM round-trip).
All dense projections run in bf16 (f32 matmuls are 4x slower on the PE).
"""
import sys

import numpy as np

try:
    import concourse  # noqa: F401
except ImportError:  # pragma: no cover
    sys.path.insert(0, "/opt/trn_rl_repo")

import ml_dtypes

import concourse.bacc as bacc
import concourse.bass as bass
import concourse.tile as tile
from concourse import mybir
from concourse.bass_utils import run_bass_kernel_spmd

f32 = mybir.dt.float32
bf16 = mybir.dt.bfloat16
fp8 = mybir.dt.float8e4
i32 = mybir.dt.int32
AF = mybir.ActivationFunctionType
ALU = mybir.AluOpType
BF = ml_dtypes.bfloat16

FULL_CFG = dict(N=100000, E=500000, HID=128, H=4, D=32, L=2, NC=8)


def _blockdiag(a):
    H, D, _ = a.shape
    out = np.zeros((H * D, H * D), np.float32)
    for h in range(H):
        out[h * D:(h + 1) * D, h * D:(h + 1) * D] = a[h]
    return out


def _balance_windows(deg, W, cap=640):
    """LPT-balance nodes into W windows of <=128 slots, then push excess
    above `cap` edges into the single overflow window W-1 via degree swaps."""
    import heapq
    n = len(deg)
    order = np.argsort(-deg, kind="stable")
    heap = [(0, w) for w in range(W)]
    heapq.heapify(heap)
    slots_used = np.zeros(W, np.int64)
    edge_cnt = np.zeros(W, np.int64)
    assign = np.empty(n, np.int64)
    for nd in order:
        while True:
            cnt, w = heapq.heappop(heap)
            if cnt == edge_cnt[w] and slots_used[w] < 128:
                break
        assign[nd] = w
        slots_used[w] += 1
        edge_cnt[w] += deg[nd]
        heapq.heappush(heap, (edge_cnt[w], w))
    ov = W - 1
    if W >= 2 and edge_cnt.max() > cap:
        nodes_w = [list(np.where(assign == w)[0]) for w in range(W)]
        for w in range(W):
            if w == ov:
                continue
            while edge_cnt[w] > cap:
                part = min(nodes_w[ov], key=lambda nd: deg[nd])
                need = edge_cnt[w] - cap + deg[part]
                cands = [nd for nd in nodes_w[w] if deg[nd] >= need]
                cand = (min(cands, key=lambda nd: deg[nd]) if cands
                        else max(nodes_w[w], key=lambda nd: deg[nd]))
                if deg[cand] <= deg[part]:
                    break
                nodes_w[w].remove(cand)
                nodes_w[ov].remove(part)
                nodes_w[w].append(part)
                nodes_w[ov].append(cand)
                edge_cnt[w] += deg[part] - deg[cand]
                edge_cnt[ov] += deg[cand] - deg[part]
        for w in range(W):
            for nd in nodes_w[w]:
                assign[nd] = w
    slot_in_w = np.zeros(n, np.int64)
    fill = np.zeros(W, np.int64)
    for nd in range(n):
        w = assign[nd]
        slot_in_w[nd] = fill[w]
        fill[w] += 1
    return assign * 128 + slot_in_w


def host_prep(inputs, cfg):
    N, E, HID, H, D, L, NC = (cfg[k] for k in ("N", "E", "HID", "H", "D", "L", "NC"))
    NSH = N // NC
    W = (NSH + 127) // 128
    NP = W * 128

    ip = {k: np.asarray(v) for k, v in inputs.items()}
    rel_st = [0, 1]
    rel_dt = [1, 0]
    edges = [ip["edge_ui"], ip["edge_iu"]]

    scale = 1.0 / np.sqrt(D)
    W3 = np.zeros((L, 2, HID, 3 * HID), np.float32)
    b3 = np.zeros((L, 2, 3 * HID), np.float32)
    for l in range(L):
        for r in range(2):
            st = rel_st[r]
            BDa = _blockdiag(ip["a_rel"][l, r])
            BDm = _blockdiag(ip["m_rel"][l, r])
            W3[l, r, :, 0:HID] = ip["Wk"][l, st] @ BDa
            b3[l, r, 0:HID] = ip["bk"][l, st] @ BDa
            W3[l, r, :, HID:2 * HID] = ip["Wv"][l, st] @ BDm
            b3[l, r, HID:2 * HID] = ip["bv"][l, st] @ BDm
        for t in range(2):
            r_of = 1 - t
            pscale = np.repeat(ip["p_rel"][l, r_of] * scale, D)
            W3[l, t, :, 2 * HID:3 * HID] = ip["Wq"][l, t] * pscale[None, :]
            b3[l, t, 2 * HID:3 * HID] = ip["bq"][l, t] * pscale
    beta = 1.0 / (1.0 + np.exp(-ip["skip"]))

    r_of_dt = [1, 0]
    newslot = np.zeros((2, NC, NSH), np.int64)
    for t in range(2):
        r = r_of_dt[t]
        dst = edges[r][1].astype(np.int64)
        deg_all = np.bincount(dst, minlength=N)
        for c in range(NC):
            newslot[t, c] = _balance_windows(deg_all[c * NSH:(c + 1) * NSH], W)
    orderv = np.full((2, NC, NP), -1, np.int64)
    for t in range(2):
        for c in range(NC):
            orderv[t, c, newslot[t, c]] = np.arange(NSH)

    def prep_rel(r):
        e = edges[r]
        st, dt = rel_st[r], rel_dt[r]
        src, dst = e[0].astype(np.int64), e[1].astype(np.int64)
        csrc = src // NSH
        gsrc = csrc * NP + newslot[st][csrc, src % NSH]
        counts = np.zeros((NC, W), np.int64)
        per_core = []
        for c in range(NC):
            sel = (dst // NSH) == c
            s_c = gsrc[sel]
            dl_c = newslot[dt, c][dst[sel] - c * NSH]
            order = np.argsort(dl_c, kind="stable")
            s_c, dl_c = s_c[order], dl_c[order]
            counts[c] = np.bincount(dl_c // 128, minlength=W)
            per_core.append((s_c, dl_c))
        tiles_w = np.maximum(1, (counts.max(axis=0) + 127) // 128)
        NT = int(tiles_w.sum())
        idx_src = np.zeros((NC, NT * 128), np.int32)
        dloc = np.full((NC, NT * 128), 128.0, np.float32)
        for c in range(NC):
            s_c, dl_c = per_core[c]
            starts = np.concatenate([[0], np.cumsum(counts[c])])
            slot = 0
            for w in range(W):
                n = int(counts[c][w])
                a, b = int(starts[w]), int(starts[w]) + n
                idx_src[c, slot:slot + n] = s_c[a:b]
                dloc[c, slot:slot + n] = (dl_c[a:b] % 128).astype(np.float32)
                slot += int(tiles_w[w]) * 128
        return tiles_w, NT, idx_src, dloc

    schedules, meta = [], []
    for r in range(2):
        tiles_w, NT, idx_src, dloc = prep_rel(r)
        schedules.append((tiles_w, NT))
        meta.append((idx_src, dloc))

    NTtot = schedules[0][1] + schedules[1][1]
    W2 = 2 * W
    NTWMAX = max(int(t) for s in schedules for t in s[0])
    xs = [ip["x_user"].astype(np.float32), ip["x_item"].astype(np.float32)]
    in_maps = []
    for c in range(NC):
        x_fm = np.zeros((2, HID, NP), np.float32)
        for t in range(2):
            x_fm[t][:, newslot[t, c]] = xs[t][c * NSH:(c + 1) * NSH].T
        idx_cat = np.concatenate(
            [meta[0][0][c], meta[1][0][c]]).reshape(NTtot, 128).T
        dl = np.concatenate([meta[0][1][c], meta[1][1][c]])
        eye129 = np.zeros((129, 128), np.float32)
        eye129[:128] = np.eye(128, dtype=np.float32)
        A = eye129[dl.astype(np.int64)].reshape(NTtot, 128, 128)
        S2_all = np.ascontiguousarray(
            A.transpose(1, 0, 2).reshape(128, NTtot * 128).astype(BF))
        St_all = np.ascontiguousarray(
            A.transpose(2, 0, 1).reshape(128, NTtot * 128).astype(BF))
        in_maps.append({
            "x_fm": x_fm.astype(BF),
            "idx_src": np.ascontiguousarray(idx_cat.astype(np.int32)),
            "S2_all": S2_all,
            "St_all": St_all,
        })

    b_list = [ip["b_in"][0], ip["b_in"][1]]
    for l in range(L):
        for t in range(2):
            b_list.append(beta[l, t] * ip["bo"][l, t])
    Bcols = np.stack(b_list).astype(np.float32)

    bias_nz = [[bool(np.any(b3[l, r] != 0)) for r in range(2)] for l in range(L)]
    consts = {
        "bias_nz": bias_nz,
        "iota_row": np.tile(np.arange(128, dtype=np.float32), (128, 1)).astype(BF),
        "ident": np.eye(128, dtype=np.float32).astype(BF),
        "W3": W3.reshape(L * 2, HID, 3 * HID).astype(BF),
        "Win": ip["W_in"].astype(np.float32).astype(BF),
        "Wo_bf": ip["Wo"].astype(np.float32).reshape(L * 2, HID, HID).astype(BF),
        "b3": b3.reshape(1, L * 2 * 3 * HID).astype(np.float32),
        "Bcols": Bcols,
        "ones1f": np.ones((1, 128), np.float32).astype(BF),
    }
    dims = dict(NSH=NSH, W=W, NP=NP, NTtot=NTtot, NTWMAX=NTWMAX, W2=W2)
    return in_maps, consts, orderv, schedules, dims, beta


def build_program(cfg, consts, schedules, dims, beta, sim_gelu=False):
    N, E, HID, H, D, L, NC = (cfg[k] for k in ("N", "E", "HID", "H", "D", "L", "NC"))
    NSH, W, NP, NTtot = dims["NSH"], dims["W"], dims["NP"], dims["NTtot"]
    NTWMAX, W2 = dims["NTWMAX"], dims["W2"]
    NPALL = NP * NC
    rel_dt = [1, 0]
    NB = consts["Bcols"].shape[0]
    CHD = 448 if NP % 448 == 0 else 128
    CHN = 896 if NP % 896 == 0 else (512 if NP % 512 == 0 else NP)
    assert NP % CHD == 0 and NP % CHN == 0 and CHN % 128 == 0

    nc = bacc.Bacc("TRN2", target_bir_lowering=False, debug=False,
                   num_devices=NC)

    x_fm = nc.dram_tensor("x_fm", [2, HID, NP], bf16, kind="ExternalInput")
    idx_src = nc.dram_tensor("idx_src", [128, NTtot], i32, kind="ExternalInput")
    S2_d = nc.dram_tensor("S2_all", [128, NTtot * 128], bf16,
                          kind="ExternalInput")
    St_d = nc.dram_tensor("St_all", [128, NTtot * 128], bf16,
                          kind="ExternalInput")
    ident_d = nc.dram_tensor("ident", [128, 128], bf16, kind="ExternalInput")
    W3_d = nc.dram_tensor("W3", [L * 2, HID, 3 * HID], bf16, kind="ExternalInput")
    Win_d = nc.dram_tensor("Win", [2, HID, HID], bf16, kind="ExternalInput")
    Wo_d = nc.dram_tensor("Wo_bf", [L * 2, HID, HID], bf16, kind="ExternalInput")
    b3_d = nc.dram_tensor("b3", [1, L * 2 * 3 * HID], f32, kind="ExternalInput")
    Bcols_d = nc.dram_tensor("Bcols", [NB, HID], f32, kind="ExternalInput")
    ones1f_d = nc.dram_tensor("ones1f", [1, 128], bf16, kind="ExternalInput")
    out_d = nc.dram_tensor("out", [2, HID, NP], f32, kind="ExternalOutput")

    with tile.TileContext(nc) as tc:
        with tc.tile_pool(name="persist", bufs=1) as pp, \
             tc.tile_pool(name="dram", bufs=1, space="DRAM") as dp, \
             tc.tile_pool(name="wk_sb", bufs=3) as sb3, \
             tc.tile_pool(name="wk_sb2", bufs=2) as sb2, \
             tc.tile_pool(name="gath", bufs=3) as gpool, \
             tc.tile_pool(name="edge8", bufs=3) as sb8, \
             tc.tile_pool(name="ps_win", bufs=2, space="PSUM") as ps_w, \
             tc.tile_pool(name="ps_qe", bufs=2, space="PSUM") as ps_q, \
             tc.tile_pool(name="ps_dense", bufs=2, space="PSUM") as ps_d:

            ident = pp.tile([128, 128], bf16)
            nc.sync.dma_start(ident[:], ident_d[:])
            onesf = pp.tile([1, 128], bf16)
            nc.sync.dma_start(onesf[:], ones1f_d[:])
            idxs = pp.tile([128, NTtot], i32)
            nc.sync.dma_start(idxs[:], idx_src[:])
            w3sb = pp.tile([128, L * 2, 3 * HID], bf16)
            nc.sync.dma_start(w3sb[:], W3_d[:].rearrange("k p d -> p k d"))
            winsb = pp.tile([128, 2, HID], bf16)
            nc.sync.dma_start(winsb[:], Win_d[:].rearrange("k p d -> p k d"))
            wosb = pp.tile([128, L * 2, HID], bf16)
            nc.sync.dma_start(wosb[:], Wo_d[:].rearrange("k p d -> p k d"))
            b3sb = pp.tile([1, L * 2 * 3 * HID], f32)
            nc.sync.dma_start(b3sb[:], b3_d[:])
            bcols = pp.tile([128, NB], f32)
            nc.sync.dma_start(bcols[:], Bcols_d[:].rearrange("k d -> d k"))

            g_fm = [pp.tile([128, NP], bf16, name=f"g_fm{t}") for t in range(2)]
            q_sb = [pp.tile([128, W, 128], bf16, name=f"q_sb{t}")
                    for t in range(2)]

            hA = [dp.tile([128, NP], bf16, name=f"hA{t}") for t in range(2)]
            hB = [dp.tile([128, NP], bf16, name=f"hB{t}") for t in range(2)]
            kvloc = [dp.tile([NP, 256], fp8, name=f"kvloc{r}") for r in range(2)]
            kvfull = [[dp.tile([NPALL, 256], fp8, name=f"kvfull{l}{r}")
                       for r in range(2)] for l in range(L)]
            rg = [list(range(NC))]

            def b3row(l, r, lo, hi):
                base = (l * 2 + r) * 3 * HID
                return b3sb[:, base + lo:base + hi]

            def node_pass(l, r, h_src):
                for jc in range(NP // CHN):
                    hch = sb3.tile([128, CHN], bf16, tag="hch")
                    nc.sync.dma_start(hch[:], h_src[:, jc * CHN:(jc + 1) * CHN])
                    for k in range(CHN // 128):
                        w = jc * (CHN // 128) + k
                        ps = ps_d.tile([128, 3 * HID], f32, tag="dense")
                        bias_nz = consts["bias_nz"][l][r]
                        nc.tensor.matmul(
                            out=ps[:], lhsT=hch[:, k * 128:(k + 1) * 128],
                            rhs=w3sb[:, l * 2 + r, :], start=True,
                            stop=not bias_nz)
                        if bias_nz:
                            nc.tensor.matmul(
                                out=ps[:], lhsT=onesf[:],
                                rhs=b3row(l, r, 0, 3 * HID), start=False,
                                stop=True)
                        kv8 = sb3.tile([128, 256], fp8, tag="kv8")
                        nc.scalar.activation(kv8[:], ps[:, 0:256], AF.Copy)
                        nc.sync.dma_start(
                            kvloc[r][w * 128:(w + 1) * 128, :], kv8[:])
                        nc.vector.tensor_copy(q_sb[r][:, w, :],
                                              ps[:, 256:384])

            def edge_phase(l, r, tbase, wbase):
                tiles_w, NT = schedules[r]
                dt = rel_dt[r]
                t0 = tbase
                for w in range(W):
                    nt = int(tiles_w[w])
                    kvg = gpool.tile([128, NTWMAX, 256], fp8, tag="kv")
                    for i in range(nt):
                        nc.gpsimd.indirect_dma_start(
                            out=kvg[:, i, :], out_offset=None,
                            in_=kvfull[l][r][:],
                            in_offset=bass.IndirectOffsetOnAxis(
                                ap=idxs[:, t0 + i:t0 + i + 1], axis=0))
                    S2 = sb8.tile([128, NTWMAX, 128], bf16, tag="S")
                    nc.sync.dma_start(
                        S2[:, 0:nt, :].rearrange("p a b -> p (a b)"),
                        S2_d[:, t0 * 128:(t0 + nt) * 128])
                    St = sb8.tile([128, NTWMAX, 128], bf16, tag="St")
                    nc.sync.dma_start(
                        St[:, 0:nt, :].rearrange("p a b -> p (a b)"),
                        St_d[:, t0 * 128:(t0 + nt) * 128])
                    pay = sb8.tile([128, NTWMAX, 132], bf16, tag="pay")
                    k2 = 0
                    while k2 < nt:
                        g = min(2, nt - k2)
                        psqe = ps_q.tile([128, 2, 128], f32, tag="qe")
                        for i in range(g):
                            nc.tensor.matmul(out=psqe[:, i, :],
                                             lhsT=St[:, k2 + i, :],
                                             rhs=q_sb[dt][:, w, :],
                                             start=True, stop=True)
                        qk = sb8.tile([128, 2, 128], bf16, tag="qk")
                        nc.vector.tensor_tensor(
                            out=qk[:, 0:g, :], in0=psqe[:, 0:g, :],
                            in1=kvg[:, k2:k2 + g, 0:128], op=ALU.mult)
                        lg = sb8.tile([128, 2, H], f32, tag="lg")
                        nc.vector.tensor_reduce(
                            out=lg[:, 0:g, :],
                            in_=qk[:, 0:g, :].rearrange(
                                "p g (h d) -> p (g h) d", h=H),
                            axis=mybir.AxisListType.X, op=ALU.add)
                        nc.scalar.activation(pay[:, k2:k2 + g, 128:132],
                                             lg[:, 0:g, :], AF.Exp)
                        k2 += g
                    nc.vector.tensor_tensor(
                        out=pay[:, 0:nt, 0:128].rearrange(
                            "p g (h d) -> p g h d", h=H),
                        in0=kvg[:, 0:nt, 128:256].rearrange(
                            "p g (h d) -> p g h d", h=H),
                        in1=pay[:, 0:nt, 128:132].unsqueeze(3)
                            .to_broadcast([128, nt, H, D]),
                        op=ALU.mult)
                    pswin = ps_w.tile([128, 132], f32, tag="win")
                    for i in range(nt):
                        nc.tensor.matmul(out=pswin[:], lhsT=S2[:, i, :],
                                         rhs=pay[:, i, :],
                                         start=(i == 0), stop=(i == nt - 1))
                    zrw = sb8.tile([128, 4], f32, tag="zrw")
                    nc.vector.tensor_scalar(
                        out=zrw[:], in0=pswin[:, 128:132],
                        scalar1=1e-16, scalar2=None, op0=ALU.add)
                    nc.vector.reciprocal(zrw[:], zrw[:])
                    gt = sb8.tile([128, 128], bf16, tag="gt")
                    nc.vector.tensor_tensor(
                        out=gt[:].rearrange("p (h d) -> p h d", h=H),
                        in0=pswin[:, 0:128].rearrange("p (h d) -> p h d", h=H),
                        in1=zrw[:].unsqueeze(2).to_broadcast([128, H, D]),
                        op=ALU.mult)
                    psgt = ps_q.tile([128, 2, 128], bf16, tag="stt")
                    nc.tensor.transpose(out=psgt[:, 0, :], in_=gt[:],
                                        identity=ident[:])
                    nc.scalar.copy(
                        g_fm[dt][:, w * 128:(w + 1) * 128], psgt[:, 0, :])
                    t0 += nt

            def bulk_gelu(t, lo, hi):
                if not sim_gelu:
                    nc.scalar.activation(g_fm[t][:, lo:hi], g_fm[t][:, lo:hi],
                                         AF.Gelu)
                else:
                    tmp = sb2.tile([128, NP], f32, tag="sgl")
                    g = g_fm[t][:, lo:hi]
                    tm = tmp[:, lo:hi]
                    nc.vector.tensor_tensor(out=tm, in0=g, in1=g, op=ALU.mult)
                    nc.vector.tensor_scalar(out=tm, in0=tm, scalar1=0.044715,
                                            scalar2=1.0, op0=ALU.mult,
                                            op1=ALU.add)
                    nc.vector.tensor_tensor(out=tm, in0=tm, in1=g, op=ALU.mult)
                    nc.scalar.activation(tm, tm, AF.Tanh,
                                         scale=0.7978845608028654)
                    nc.vector.tensor_scalar(out=tm, in0=tm, scalar1=1.0,
                                            scalar2=0.5, op0=ALU.add,
                                            op1=ALU.mult)
                    nc.vector.tensor_tensor(out=g, in0=tm, in1=g, op=ALU.mult)

            def out_phase(l, t, h_src, dst, last):
                bb = 2 + l * 2 + t
                coef = float((1.0 - beta[l, t]) + (1.0 if l > 0 else 0.0))
                bulk_gelu(t, 0, NP)
                for j in range(NP // CHD):
                    sl = slice(j * CHD, (j + 1) * CHD)
                    ps = ps_d.tile([128, CHD], f32, tag="dense")
                    nc.tensor.matmul(out=ps[:], lhsT=wosb[:, l * 2 + t, :],
                                     rhs=g_fm[t][:, sl], start=True, stop=True)
                    a1 = sb2.tile([128, CHD], f32, tag="a1")
                    nc.vector.tensor_scalar(
                        out=a1[:], in0=ps[:], scalar1=float(beta[l, t]),
                        scalar2=bcols[:, bb:bb + 1], op0=ALU.mult, op1=ALU.add)
                    hch = sb2.tile([128, CHD], bf16, tag="hcho")
                    nc.sync.dma_start(hch[:], h_src[:, sl])
                    hn = sb2.tile([128, CHD], f32 if last else bf16,
                                  tag="hnf" if last else "hn")
                    nc.vector.scalar_tensor_tensor(
                        out=hn[:], in0=hch[:], scalar=coef, in1=a1[:],
                        op0=ALU.mult, op1=ALU.add)
                    nc.sync.dma_start(dst[:, sl], hn[:])

            def input_proj(t, dst):
                for j in range(NP // CHD):
                    sl = slice(j * CHD, (j + 1) * CHD)
                    xt = sb2.tile([128, CHD], bf16, tag="xt")
                    nc.sync.dma_start(xt[:], x_fm[t, :, sl])
                    ps = ps_d.tile([128, CHD], f32, tag="dense")
                    nc.tensor.matmul(out=ps[:], lhsT=winsb[:, t, :], rhs=xt[:],
                                     start=True, stop=True)
                    ht = sb2.tile([128, CHD], bf16, tag="ht")
                    nc.scalar.activation(ht[:], ps[:], AF.Relu,
                                         bias=bcols[:, t:t + 1], scale=1.0)
                    nc.sync.dma_start(dst[:, sl], ht[:])

            # ---------------- schedule ----------------
            rorder = [[0, 1] if l % 2 == 0 else [1, 0] for l in range(L)]
            tb = [0, schedules[0][1]]
            wb = [0, W]

            h_cur = hA
            rF, rS = rorder[0]
            input_proj(rF, hA[rF])
            node_pass(0, rF, hA[rF])
            nc.gpsimd.collective_compute(
                "AllGather", ALU.bypass, replica_groups=rg,
                ins=[kvloc[rF][:]], outs=[kvfull[0][rF][:]])
            input_proj(rS, hA[rS])
            node_pass(0, rS, hA[rS])
            nc.gpsimd.collective_compute(
                "AllGather", ALU.bypass, replica_groups=rg,
                ins=[kvloc[rS][:]], outs=[kvfull[0][rS][:]])

            for l in range(L):
                rF, rS = rorder[l]
                last = l == L - 1
                h_nxt = hB if l == 0 else None
                edge_phase(l, rF, tb[rF], wb[rF])
                tF_out = rel_dt[rF]
                dstF = (h_nxt[tF_out] if not last else out_d[tF_out])
                out_phase(l, tF_out, h_cur[tF_out], dstF, last)
                if not last:
                    l2 = l + 1
                    rF2, rS2 = rorder[l2]
                    node_pass(l2, rF2, h_nxt[rF2])
                    nc.gpsimd.collective_compute(
                        "AllGather", ALU.bypass, replica_groups=rg,
                        ins=[kvloc[rF2][:]], outs=[kvfull[l2][rF2][:]])
                edge_phase(l, rS, tb[rS], wb[rS])
                tS_out = rel_dt[rS]
                dstS = (h_nxt[tS_out] if not last else out_d[tS_out])
                out_phase(l, tS_out, h_cur[tS_out], dstS, last)
                if not last:
                    node_pass(l2, rS2, h_nxt[rS2])
                    nc.gpsimd.collective_compute(
                        "AllGather", ALU.bypass, replica_groups=rg,
                        ins=[kvloc[rS2][:]], outs=[kvfull[l2][rS2][:]])
                    h_cur = hB

    nc.finalize()
    return nc


def run(inputs, cfg=None, trace=False, trace_cores=None, sim=False):
    cfg = cfg or FULL_CFG
    NC = cfg["NC"]
    core_maps, consts, orderv, schedules, dims, beta = host_prep(inputs, cfg)
    nc = build_program(cfg, consts, schedules, dims, beta, sim_gelu=sim)
    in_maps = []
    for c in range(NC):
        m = dict(core_maps[c])
        for k in ("ident", "W3", "Win", "Wo_bf", "b3",
                  "Bcols", "ones1f"):
            m[k] = consts[k]
        in_maps.append(m)
    if sim:
        from concourse.bass_interp import MultiCoreSim

        msim = MultiCoreSim(nc, num_cores=NC, trace=False,
                            require_finite=False, require_nnan=False)
        cores = [msim.cores[c] for c in range(NC)]
        for c in range(NC):
            for name, arr in in_maps[c].items():
                cores[c].tensor(name)[:] = arr
        msim.simulate(check_with_hw=False)

        class R:
            exec_time_ns = None
            results = [{"out": np.asarray(cores[c].tensor("out"))}
                       for c in range(NC)]
        res = R()
    else:
        res = run_bass_kernel_spmd(nc, in_maps, core_ids=list(range(NC)),
                                   trace=trace, trace_cores=trace_cores)
    NSH, NP = dims["NSH"], dims["NP"]
    out = np.empty((2, cfg["N"], cfg["HID"]), np.float32)
    for c in range(NC):
        o = res.results[c]["out"]
        for t in range(2):
            sel = orderv[t, c] >= 0
            slots = np.where(sel)[0]
            orig = orderv[t, c][slots]
            out[t, c * NSH + orig] = o[t][:, slots].T
    return out, res


def kernel(**inputs):
    out, _ = run(inputs, FULL_CFG, trace=False)
    return out


# revision 8
# speedup vs baseline: 1.3996x; 1.1611x over previous
"""HGT Bass kernel for Trainium2, 8 NeuronCores — v6 (fp8 kv, streamed one-hots, deep gather run-ahead).

Dst-sharded edges with host-side balanced windows (~1% tile padding),
AllGathered bf16 kv tables, per-window segment softmax via one-hot matmuls.
Per-edge kv rows come from the proven per-tile [128,1] indirect DMA; the
per-edge q rows are expanded on the PE from SBUF-resident per-window q via
a DVE-built transposed one-hot (no PE transpose, no PSUM round-trip).
All dense projections run in bf16 (f32 matmuls are 4x slower on the PE).
"""
import sys

import numpy as np

try:
    import concourse  # noqa: F401
except ImportError:  # pragma: no cover
    sys.path.insert(0, "/opt/trn_rl_repo")

import ml_dtypes

import concourse.bacc as bacc
import concourse.bass as bass
import concourse.tile as tile
from concourse import mybir
from concourse.bass_utils import run_bass_kernel_spmd

f32 = mybir.dt.float32
bf16 = mybir.dt.bfloat16
fp8 = mybir.dt.float8e4
i32 = mybir.dt.int32
AF = mybir.ActivationFunctionType
ALU = mybir.AluOpType
BF = ml_dtypes.bfloat16

FULL_CFG = dict(N=100000, E=500000, HID=128, H=4, D=32, L=2, NC=8)


def _blockdiag(a):
    H, D, _ = a.shape
    out = np.zeros((H * D, H * D), np.float32)
    for h in range(H):
        out[h * D:(h + 1) * D, h * D:(h + 1) * D] = a[h]
    return out


def _balance_windows(deg, W, cap=640):
    """LPT-balance nodes into W windows of <=128 slots, then push excess
    above `cap` edges into the single overflow window W-1 via degree swaps."""
    import heapq
    n = len(deg)
    order = np.argsort(-deg, kind="stable")
    heap = [(0, w) for w in range(W)]
    heapq.heapify(heap)
    slots_used = np.zeros(W, np.int64)
    edge_cnt = np.zeros(W, np.int64)
    assign = np.empty(n, np.int64)
    for nd in order:
        while True:
            cnt, w = heapq.heappop(heap)
            if cnt == edge_cnt[w] and slots_used[w] < 128:
                break
        assign[nd] = w
        slots_used[w] += 1
        edge_cnt[w] += deg[nd]
        heapq.heappush(heap, (edge_cnt[w], w))
    ov = W - 1
    if W >= 2 and edge_cnt.max() > cap:
        nodes_w = [list(np.where(assign == w)[0]) for w in range(W)]
        for w in range(W):
            if w == ov:
                continue
            while edge_cnt[w] > cap:
                part = min(nodes_w[ov], key=lambda nd: deg[nd])
                need = edge_cnt[w] - cap + deg[part]
                cands = [nd for nd in nodes_w[w] if deg[nd] >= need]
                cand = (min(cands, key=lambda nd: deg[nd]) if cands
                        else max(nodes_w[w], key=lambda nd: deg[nd]))
                if deg[cand] <= deg[part]:
                    break
                nodes_w[w].remove(cand)
                nodes_w[ov].remove(part)
                nodes_w[w].append(part)
                nodes_w[ov].append(cand)
                edge_cnt[w] += deg[part] - deg[cand]
                edge_cnt[ov] += deg[cand] - deg[part]
        for w in range(W):
            for nd in nodes_w[w]:
                assign[nd] = w
    slot_in_w = np.zeros(n, np.int64)
    fill = np.zeros(W, np.int64)
    for nd in range(n):
        w = assign[nd]
        slot_in_w[nd] = fill[w]
        fill[w] += 1
    return assign * 128 + slot_in_w


def host_prep(inputs, cfg):
    N, E, HID, H, D, L, NC = (cfg[k] for k in ("N", "E", "HID", "H", "D", "L", "NC"))
    NSH = N // NC
    W = (NSH + 127) // 128
    NP = W * 128

    ip = {k: np.asarray(v) for k, v in inputs.items()}
    rel_st = [0, 1]
    rel_dt = [1, 0]
    edges = [ip["edge_ui"], ip["edge_iu"]]

    scale = 1.0 / np.sqrt(D)
    W3 = np.zeros((L, 2, HID, 3 * HID), np.float32)
    b3 = np.zeros((L, 2, 3 * HID), np.float32)
    for l in range(L):
        for r in range(2):
            st = rel_st[r]
            BDa = _blockdiag(ip["a_rel"][l, r])
            BDm = _blockdiag(ip["m_rel"][l, r])
            W3[l, r, :, 0:HID] = ip["Wk"][l, st] @ BDa
            b3[l, r, 0:HID] = ip["bk"][l, st] @ BDa
            W3[l, r, :, HID:2 * HID] = ip["Wv"][l, st] @ BDm
            b3[l, r, HID:2 * HID] = ip["bv"][l, st] @ BDm
        for t in range(2):
            r_of = 1 - t
            pscale = np.repeat(ip["p_rel"][l, r_of] * scale, D)
            W3[l, t, :, 2 * HID:3 * HID] = ip["Wq"][l, t] * pscale[None, :]
            b3[l, t, 2 * HID:3 * HID] = ip["bq"][l, t] * pscale
    beta = 1.0 / (1.0 + np.exp(-ip["skip"]))

    r_of_dt = [1, 0]
    newslot = np.zeros((2, NC, NSH), np.int64)
    for t in range(2):
        r = r_of_dt[t]
        dst = edges[r][1].astype(np.int64)
        deg_all = np.bincount(dst, minlength=N)
        for c in range(NC):
            newslot[t, c] = _balance_windows(deg_all[c * NSH:(c + 1) * NSH], W)
    orderv = np.full((2, NC, NP), -1, np.int64)
    for t in range(2):
        for c in range(NC):
            orderv[t, c, newslot[t, c]] = np.arange(NSH)

    def prep_rel(r):
        e = edges[r]
        st, dt = rel_st[r], rel_dt[r]
        src, dst = e[0].astype(np.int64), e[1].astype(np.int64)
        csrc = src // NSH
        gsrc = csrc * NP + newslot[st][csrc, src % NSH]
        counts = np.zeros((NC, W), np.int64)
        per_core = []
        for c in range(NC):
            sel = (dst // NSH) == c
            s_c = gsrc[sel]
            dl_c = newslot[dt, c][dst[sel] - c * NSH]
            order = np.argsort(dl_c, kind="stable")
            s_c, dl_c = s_c[order], dl_c[order]
            counts[c] = np.bincount(dl_c // 128, minlength=W)
            per_core.append((s_c, dl_c))
        tiles_w = np.maximum(1, (counts.max(axis=0) + 127) // 128)
        NT = int(tiles_w.sum())
        idx_src = np.zeros((NC, NT * 128), np.int32)
        dloc = np.full((NC, NT * 128), 128.0, np.float32)
        for c in range(NC):
            s_c, dl_c = per_core[c]
            starts = np.concatenate([[0], np.cumsum(counts[c])])
            slot = 0
            for w in range(W):
                n = int(counts[c][w])
                a, b = int(starts[w]), int(starts[w]) + n
                idx_src[c, slot:slot + n] = s_c[a:b]
                dloc[c, slot:slot + n] = (dl_c[a:b] % 128).astype(np.float32)
                slot += int(tiles_w[w]) * 128
        return tiles_w, NT, idx_src, dloc

    schedules, meta = [], []
    for r in range(2):
        tiles_w, NT, idx_src, dloc = prep_rel(r)
        schedules.append((tiles_w, NT))
        meta.append((idx_src, dloc))

    NTtot = schedules[0][1] + schedules[1][1]
    W2 = 2 * W
    NTWMAX = max(int(t) for s in schedules for t in s[0])
    xs = [ip["x_user"].astype(np.float32), ip["x_item"].astype(np.float32)]
    in_maps = []
    for c in range(NC):
        x_fm = np.zeros((2, HID, NP), np.float32)
        for t in range(2):
            x_fm[t][:, newslot[t, c]] = xs[t][c * NSH:(c + 1) * NSH].T
        idx_cat = np.concatenate(
            [meta[0][0][c], meta[1][0][c]]).reshape(NTtot, 128).T
        dl = np.concatenate([meta[0][1][c], meta[1][1][c]])
        eye129 = np.zeros((129, 128), np.float32)
        eye129[:128] = np.eye(128, dtype=np.float32)
        A = eye129[dl.astype(np.int64)].reshape(NTtot, 128, 128)
        S2_all = np.ascontiguousarray(
            A.transpose(1, 0, 2).reshape(128, NTtot * 128).astype(BF))
        St_all = np.ascontiguousarray(
            A.transpose(2, 0, 1).reshape(128, NTtot * 128).astype(BF))
        in_maps.append({
            "x_fm": x_fm.astype(BF),
            "idx_src": np.ascontiguousarray(idx_cat.astype(np.int32)),
            "S2_all": S2_all,
            "St_all": St_all,
        })

    b_list = [ip["b_in"][0], ip["b_in"][1]]
    for l in range(L):
        for t in range(2):
            b_list.append(beta[l, t] * ip["bo"][l, t])
    Bcols = np.stack(b_list).astype(np.float32)

    bias_nz = [[bool(np.any(b3[l, r] != 0)) for r in range(2)] for l in range(L)]
    consts = {
        "bias_nz": bias_nz,
        "iota_row": np.tile(np.arange(128, dtype=np.float32), (128, 1)).astype(BF),
        "ident": np.eye(128, dtype=np.float32).astype(BF),
        "W3": W3.reshape(L * 2, HID, 3 * HID).astype(BF),
        "Win": ip["W_in"].astype(np.float32).astype(BF),
        "Wo_bf": ip["Wo"].astype(np.float32).reshape(L * 2, HID, HID).astype(BF),
        "b3": b3.reshape(1, L * 2 * 3 * HID).astype(np.float32),
        "Bcols": Bcols,
        "ones1f": np.ones((1, 128), np.float32).astype(BF),
    }
    dims = dict(NSH=NSH, W=W, NP=NP, NTtot=NTtot, NTWMAX=NTWMAX, W2=W2)
    return in_maps, consts, orderv, schedules, dims, beta


def build_program(cfg, consts, schedules, dims, beta, sim_gelu=False):
    N, E, HID, H, D, L, NC = (cfg[k] for k in ("N", "E", "HID", "H", "D", "L", "NC"))
    NSH, W, NP, NTtot = dims["NSH"], dims["W"], dims["NP"], dims["NTtot"]
    NTWMAX, W2 = dims["NTWMAX"], dims["W2"]
    NPALL = NP * NC
    rel_dt = [1, 0]
    NB = consts["Bcols"].shape[0]
    CHD = 448 if NP % 448 == 0 else 128
    CHN = 896 if NP % 896 == 0 else (512 if NP % 512 == 0 else NP)
    assert NP % CHD == 0 and NP % CHN == 0 and CHN % 128 == 0

    nc = bacc.Bacc("TRN2", target_bir_lowering=False, debug=False,
                   num_devices=NC)

    x_fm = nc.dram_tensor("x_fm", [2, HID, NP], bf16, kind="ExternalInput")
    idx_src = nc.dram_tensor("idx_src", [128, NTtot], i32, kind="ExternalInput")
    S2_d = nc.dram_tensor("S2_all", [128, NTtot * 128], bf16,
                          kind="ExternalInput")
    St_d = nc.dram_tensor("St_all", [128, NTtot * 128], bf16,
                          kind="ExternalInput")
    ident_d = nc.dram_tensor("ident", [128, 128], bf16, kind="ExternalInput")
    W3_d = nc.dram_tensor("W3", [L * 2, HID, 3 * HID], bf16, kind="ExternalInput")
    Win_d = nc.dram_tensor("Win", [2, HID, HID], bf16, kind="ExternalInput")
    Wo_d = nc.dram_tensor("Wo_bf", [L * 2, HID, HID], bf16, kind="ExternalInput")
    b3_d = nc.dram_tensor("b3", [1, L * 2 * 3 * HID], f32, kind="ExternalInput")
    Bcols_d = nc.dram_tensor("Bcols", [NB, HID], f32, kind="ExternalInput")
    ones1f_d = nc.dram_tensor("ones1f", [1, 128], bf16, kind="ExternalInput")
    out_d = nc.dram_tensor("out", [2, HID, NP], f32, kind="ExternalOutput")

    with tile.TileContext(nc) as tc:
        with tc.tile_pool(name="persist", bufs=1) as pp, \
             tc.tile_pool(name="dram", bufs=1, space="DRAM") as dp, \
             tc.tile_pool(name="wk_sb", bufs=3) as sb3, \
             tc.tile_pool(name="wk_sb2", bufs=2) as sb2, \
             tc.tile_pool(name="gath", bufs=16) as gpool, \
             tc.tile_pool(name="edge8", bufs=4) as sb8, \
             tc.tile_pool(name="ps_win", bufs=2, space="PSUM") as ps_w, \
             tc.tile_pool(name="ps_qe", bufs=2, space="PSUM") as ps_q, \
             tc.tile_pool(name="ps_dense", bufs=2, space="PSUM") as ps_d:

            ident = pp.tile([128, 128], bf16)
            nc.sync.dma_start(ident[:], ident_d[:])
            onesf = pp.tile([1, 128], bf16)
            nc.sync.dma_start(onesf[:], ones1f_d[:])
            idxs = pp.tile([128, NTtot], i32)
            nc.sync.dma_start(idxs[:], idx_src[:])
            w3sb = pp.tile([128, L * 2, 3 * HID], bf16)
            nc.sync.dma_start(w3sb[:], W3_d[:].rearrange("k p d -> p k d"))
            winsb = pp.tile([128, 2, HID], bf16)
            nc.sync.dma_start(winsb[:], Win_d[:].rearrange("k p d -> p k d"))
            wosb = pp.tile([128, L * 2, HID], bf16)
            nc.sync.dma_start(wosb[:], Wo_d[:].rearrange("k p d -> p k d"))
            b3sb = pp.tile([1, L * 2 * 3 * HID], f32)
            nc.sync.dma_start(b3sb[:], b3_d[:])
            bcols = pp.tile([128, NB], f32)
            nc.sync.dma_start(bcols[:], Bcols_d[:].rearrange("k d -> d k"))

            g_fm = [pp.tile([128, NP], bf16, name=f"g_fm{t}") for t in range(2)]
            q_sb = [pp.tile([128, W, 128], bf16, name=f"q_sb{t}")
                    for t in range(2)]

            hA = [dp.tile([128, NP], bf16, name=f"hA{t}") for t in range(2)]
            hB = [dp.tile([128, NP], bf16, name=f"hB{t}") for t in range(2)]
            kvloc = [dp.tile([NP, 256], fp8, name=f"kvloc{r}") for r in range(2)]
            kvfull = [[dp.tile([NPALL, 256], fp8, name=f"kvfull{l}{r}")
                       for r in range(2)] for l in range(L)]
            rg = [list(range(NC))]

            def b3row(l, r, lo, hi):
                base = (l * 2 + r) * 3 * HID
                return b3sb[:, base + lo:base + hi]

            def node_pass(l, r, h_src):
                for jc in range(NP // CHN):
                    hch = sb3.tile([128, CHN], bf16, tag="hch")
                    nc.sync.dma_start(hch[:], h_src[:, jc * CHN:(jc + 1) * CHN])
                    for k in range(CHN // 128):
                        w = jc * (CHN // 128) + k
                        ps = ps_d.tile([128, 3 * HID], f32, tag="dense")
                        bias_nz = consts["bias_nz"][l][r]
                        nc.tensor.matmul(
                            out=ps[:], lhsT=hch[:, k * 128:(k + 1) * 128],
                            rhs=w3sb[:, l * 2 + r, :], start=True,
                            stop=not bias_nz)
                        if bias_nz:
                            nc.tensor.matmul(
                                out=ps[:], lhsT=onesf[:],
                                rhs=b3row(l, r, 0, 3 * HID), start=False,
                                stop=True)
                        kv8 = sb3.tile([128, 256], fp8, tag="kv8")
                        nc.scalar.activation(kv8[:], ps[:, 0:256], AF.Copy)
                        nc.sync.dma_start(
                            kvloc[r][w * 128:(w + 1) * 128, :], kv8[:])
                        nc.vector.tensor_copy(q_sb[r][:, w, :],
                                              ps[:, 256:384])

            def edge_phase(l, r, tbase, wbase):
                tiles_w, NT = schedules[r]
                dt = rel_dt[r]
                t0 = tbase
                for w in range(W):
                    nt = int(tiles_w[w])
                    kvg = gpool.tile([128, NTWMAX, 256], fp8, tag="kv")
                    for i in range(nt):
                        nc.gpsimd.indirect_dma_start(
                            out=kvg[:, i, :], out_offset=None,
                            in_=kvfull[l][r][:],
                            in_offset=bass.IndirectOffsetOnAxis(
                                ap=idxs[:, t0 + i:t0 + i + 1], axis=0))
                    S2 = sb8.tile([128, NTWMAX, 128], bf16, tag="S")
                    nc.sync.dma_start(
                        S2[:, 0:nt, :].rearrange("p a b -> p (a b)"),
                        S2_d[:, t0 * 128:(t0 + nt) * 128])
                    St = sb8.tile([128, NTWMAX, 128], bf16, tag="St")
                    nc.sync.dma_start(
                        St[:, 0:nt, :].rearrange("p a b -> p (a b)"),
                        St_d[:, t0 * 128:(t0 + nt) * 128])
                    pay = sb8.tile([128, NTWMAX, 132], bf16, tag="pay")
                    k2 = 0
                    while k2 < nt:
                        g = min(2, nt - k2)
                        psqe = ps_q.tile([128, 2, 128], f32, tag="qe")
                        for i in range(g):
                            nc.tensor.matmul(out=psqe[:, i, :],
                                             lhsT=St[:, k2 + i, :],
                                             rhs=q_sb[dt][:, w, :],
                                             start=True, stop=True)
                        qk = sb8.tile([128, 2, 128], bf16, tag="qk")
                        nc.vector.tensor_tensor(
                            out=qk[:, 0:g, :], in0=psqe[:, 0:g, :],
                            in1=kvg[:, k2:k2 + g, 0:128], op=ALU.mult)
                        lg = sb8.tile([128, 2, H], f32, tag="lg")
                        nc.vector.tensor_reduce(
                            out=lg[:, 0:g, :],
                            in_=qk[:, 0:g, :].rearrange(
                                "p g (h d) -> p (g h) d", h=H),
                            axis=mybir.AxisListType.X, op=ALU.add)
                        nc.scalar.activation(pay[:, k2:k2 + g, 128:132],
                                             lg[:, 0:g, :], AF.Exp)
                        k2 += g
                    nc.vector.tensor_tensor(
                        out=pay[:, 0:nt, 0:128].rearrange(
                            "p g (h d) -> p g h d", h=H),
                        in0=kvg[:, 0:nt, 128:256].rearrange(
                            "p g (h d) -> p g h d", h=H),
                        in1=pay[:, 0:nt, 128:132].unsqueeze(3)
                            .to_broadcast([128, nt, H, D]),
                        op=ALU.mult)
                    pswin = ps_w.tile([128, 132], f32, tag="win")
                    for i in range(nt):
                        nc.tensor.matmul(out=pswin[:], lhsT=S2[:, i, :],
                                         rhs=pay[:, i, :],
                                         start=(i == 0), stop=(i == nt - 1))
                    zrw = sb8.tile([128, 4], f32, tag="zrw")
                    nc.vector.tensor_scalar(
                        out=zrw[:], in0=pswin[:, 128:132],
                        scalar1=1e-16, scalar2=None, op0=ALU.add)
                    nc.vector.reciprocal(zrw[:], zrw[:])
                    gt = sb8.tile([128, 128], bf16, tag="gt")
                    nc.vector.tensor_tensor(
                        out=gt[:].rearrange("p (h d) -> p h d", h=H),
                        in0=pswin[:, 0:128].rearrange("p (h d) -> p h d", h=H),
                        in1=zrw[:].unsqueeze(2).to_broadcast([128, H, D]),
                        op=ALU.mult)
                    psgt = ps_q.tile([128, 2, 128], bf16, tag="stt")
                    nc.tensor.transpose(out=psgt[:, 0, :], in_=gt[:],
                                        identity=ident[:])
                    nc.scalar.copy(
                        g_fm[dt][:, w * 128:(w + 1) * 128], psgt[:, 0, :])
                    t0 += nt

            def bulk_gelu(t, lo, hi):
                if not sim_gelu:
                    nc.scalar.activation(g_fm[t][:, lo:hi], g_fm[t][:, lo:hi],
                                         AF.Gelu)
                else:
                    tmp = sb2.tile([128, NP], f32, tag="sgl")
                    g = g_fm[t][:, lo:hi]
                    tm = tmp[:, lo:hi]
                    nc.vector.tensor_tensor(out=tm, in0=g, in1=g, op=ALU.mult)
                    nc.vector.tensor_scalar(out=tm, in0=tm, scalar1=0.044715,
                                            scalar2=1.0, op0=ALU.mult,
                                            op1=ALU.add)
                    nc.vector.tensor_tensor(out=tm, in0=tm, in1=g, op=ALU.mult)
                    nc.scalar.activation(tm, tm, AF.Tanh,
                                         scale=0.7978845608028654)
                    nc.vector.tensor_scalar(out=tm, in0=tm, scalar1=1.0,
                                            scalar2=0.5, op0=ALU.add,
                                            op1=ALU.mult)
                    nc.vector.tensor_tensor(out=g, in0=tm, in1=g, op=ALU.mult)

            def out_phase(l, t, h_src, dst, last):
                bb = 2 + l * 2 + t
                coef = float((1.0 - beta[l, t]) + (1.0 if l > 0 else 0.0))
                bulk_gelu(t, 0, NP)
                for j in range(NP // CHD):
                    sl = slice(j * CHD, (j + 1) * CHD)
                    ps = ps_d.tile([128, CHD], f32, tag="dense")
                    nc.tensor.matmul(out=ps[:], lhsT=wosb[:, l * 2 + t, :],
                                     rhs=g_fm[t][:, sl], start=True, stop=True)
                    a1 = sb2.tile([128, CHD], f32, tag="a1")
                    nc.vector.tensor_scalar(
                        out=a1[:], in0=ps[:], scalar1=float(beta[l, t]),
                        scalar2=bcols[:, bb:bb + 1], op0=ALU.mult, op1=ALU.add)
                    hch = sb2.tile([128, CHD], bf16, tag="hcho")
                    nc.sync.dma_start(hch[:], h_src[:, sl])
                    hn = sb2.tile([128, CHD], f32 if last else bf16,
                                  tag="hnf" if last else "hn")
                    nc.vector.scalar_tensor_tensor(
                        out=hn[:], in0=hch[:], scalar=coef, in1=a1[:],
                        op0=ALU.mult, op1=ALU.add)
                    nc.sync.dma_start(dst[:, sl], hn[:])

            def input_proj(t, dst):
                for j in range(NP // CHD):
                    sl = slice(j * CHD, (j + 1) * CHD)
                    xt = sb2.tile([128, CHD], bf16, tag="xt")
                    nc.sync.dma_start(xt[:], x_fm[t, :, sl])
                    ps = ps_d.tile([128, CHD], f32, tag="dense")
                    nc.tensor.matmul(out=ps[:], lhsT=winsb[:, t, :], rhs=xt[:],
                                     start=True, stop=True)
                    ht = sb2.tile([128, CHD], bf16, tag="ht")
                    nc.scalar.activation(ht[:], ps[:], AF.Relu,
                                         bias=bcols[:, t:t + 1], scale=1.0)
                    nc.sync.dma_start(dst[:, sl], ht[:])

            # ---------------- schedule ----------------
            rorder = [[0, 1] if l % 2 == 0 else [1, 0] for l in range(L)]
            tb = [0, schedules[0][1]]
            wb = [0, W]

            h_cur = hA
            rF, rS = rorder[0]
            input_proj(rF, hA[rF])
            node_pass(0, rF, hA[rF])
            nc.gpsimd.collective_compute(
                "AllGather", ALU.bypass, replica_groups=rg,
                ins=[kvloc[rF][:]], outs=[kvfull[0][rF][:]])
            input_proj(rS, hA[rS])
            node_pass(0, rS, hA[rS])
            nc.gpsimd.collective_compute(
                "AllGather", ALU.bypass, replica_groups=rg,
                ins=[kvloc[rS][:]], outs=[kvfull[0][rS][:]])

            for l in range(L):
                rF, rS = rorder[l]
                last = l == L - 1
                h_nxt = hB if l == 0 else None
                edge_phase(l, rF, tb[rF], wb[rF])
                tF_out = rel_dt[rF]
                dstF = (h_nxt[tF_out] if not last else out_d[tF_out])
                out_phase(l, tF_out, h_cur[tF_out], dstF, last)
                if not last:
                    l2 = l + 1
                    rF2, rS2 = rorder[l2]
                    node_pass(l2, rF2, h_nxt[rF2])
                    nc.gpsimd.collective_compute(
                        "AllGather", ALU.bypass, replica_groups=rg,
                        ins=[kvloc[rF2][:]], outs=[kvfull[l2][rF2][:]])
                edge_phase(l, rS, tb[rS], wb[rS])
                tS_out = rel_dt[rS]
                dstS = (h_nxt[tS_out] if not last else out_d[tS_out])
                out_phase(l, tS_out, h_cur[tS_out], dstS, last)
                if not last:
                    node_pass(l2, rS2, h_nxt[rS2])
                    nc.gpsimd.collective_compute(
                        "AllGather", ALU.bypass, replica_groups=rg,
                        ins=[kvloc[rS2][:]], outs=[kvfull[l2][rS2][:]])
                    h_cur = hB

    nc.finalize()
    return nc


def run(inputs, cfg=None, trace=False, trace_cores=None, sim=False):
    cfg = cfg or FULL_CFG
    NC = cfg["NC"]
    core_maps, consts, orderv, schedules, dims, beta = host_prep(inputs, cfg)
    nc = build_program(cfg, consts, schedules, dims, beta, sim_gelu=sim)
    in_maps = []
    for c in range(NC):
        m = dict(core_maps[c])
        for k in ("ident", "W3", "Win", "Wo_bf", "b3",
                  "Bcols", "ones1f"):
            m[k] = consts[k]
        in_maps.append(m)
    if sim:
        from concourse.bass_interp import MultiCoreSim

        msim = MultiCoreSim(nc, num_cores=NC, trace=False,
                            require_finite=False, require_nnan=False)
        cores = [msim.cores[c] for c in range(NC)]
        for c in range(NC):
            for name, arr in in_maps[c].items():
                cores[c].tensor(name)[:] = arr
        msim.simulate(check_with_hw=False)

        class R:
            exec_time_ns = None
            results = [{"out": np.asarray(cores[c].tensor("out"))}
                       for c in range(NC)]
        res = R()
    else:
        res = run_bass_kernel_spmd(nc, in_maps, core_ids=list(range(NC)),
                                   trace=trace, trace_cores=trace_cores)
    NSH, NP = dims["NSH"], dims["NP"]
    out = np.empty((2, cfg["N"], cfg["HID"]), np.float32)
    for c in range(NC):
        o = res.results[c]["out"]
        for t in range(2):
            sel = orderv[t, c] >= 0
            slots = np.where(sel)[0]
            orig = orderv[t, c][slots]
            out[t, c * NSH + orig] = o[t][:, slots].T
    return out, res


def kernel(**inputs):
    out, _ = run(inputs, FULL_CFG, trace=False)
    return out
